# revision 1
# baseline (speedup 1.0000x reference)
"""CRF forward (partition function) kernel for Trainium2, 8 NeuronCores.

Meet-in-the-middle formulation (exp space), data-parallel over batch:
  forward   F_{i+1} = ef_i * (W @ F_i),            i = 0..M-1   (alpha side)
  backward  G_t = W^T @ (ef_t * G_{t+1}) + 1[length==t] * exp(trans[END]),
run from both ends to the midpoint M = S/2 (lengths >= S/2, so the forward
half is mask-free); host combines out[b] = log(F_M . G_M) + accumulators.

W[next,prev] = exp(trans[next,prev]); ef is exp(feat - max_tag feat) (host
prescale, bookkept via cumsum); every NK steps the device renormalizes each
batch column by r ~ 1/colsum (computed on-device, applied to a later ef
slice, exact r values dumped for host compensation).

The backward injection rides inside the one matmul per step: the state is
augmented with 3 extra rows -- row 64 a self-perpetuating constant 1, rows
65/66 per-tag-group injection markers delivered via the ef stream (marker
row at time t = 1[length==t]); the stationary has columns that (a) copy the
constant row forward and (b) add exp(trans[END])[prev] * marker to each
group's state rows.  No extra instructions, no PSUM read-modify-write.

Layout per chain: 2 tag-groups of 32 tags stacked on partitions, 64 batch
elems on the free dim; one chain per direction (forward 64 partitions,
backward 67).  The serial critical path per chain step is the PE->DVE
semaphore round trip (~500ns); the two chains interleave on the engines.
"""

import os
import sys

import numpy as np
import ml_dtypes

if "/opt/trn_rl_repo" not in sys.path:
    sys.path.insert(0, "/opt/trn_rl_repo")

import concourse.bass as bass
import concourse.tile as tile
from concourse import bacc, mybir
from concourse.bass_utils import run_bass_kernel_spmd

BF = ml_dtypes.bfloat16
S, B, T = 1024, 1024, 32
START, END = T - 2, T - 1
NCORES = 8
BC = B // NCORES            # batch per core (128)
NK, EV0, LAG = 16, 4, 6     # renorm cadence / first event / apply lag
CHUNK = 128                 # steps per DMA chunk
P, NGRP, FD = 64, 2, 64     # partitions (tags), tag groups, batch free dim
PB = P + 3                  # backward partitions (+const row, +2 markers)

dt = mybir.dt


def build_program(s_len=S):
    """One SPMD program for all cores: forward + backward half-chains."""
    m = s_len // 2
    chunk = min(CHUNK, m)
    n_ev = (m - EV0 - 1) // NK + 1 if m > EV0 else 0

    nc = bacc.Bacc("TRN2", target_bir_lowering=False, num_devices=NCORES)

    efF_d = nc.dram_tensor("efF", [P, m * FD], dt.bfloat16, kind="ExternalInput")
    efB_d = nc.dram_tensor("efB", [PB, m * FD], dt.bfloat16, kind="ExternalInput")
    y0_d = nc.dram_tensor("y0", [PB, FD], dt.bfloat16, kind="ExternalInput")
    qiF_d = nc.dram_tensor("qinitF", [P, FD], dt.bfloat16, kind="ExternalInput")
    wF_d = nc.dram_tensor("wblkF", [P, P], dt.bfloat16, kind="ExternalInput")
    wB_d = nc.dram_tensor("wblkB", [PB, PB], dt.bfloat16, kind="ExternalInput")
    obF_d = nc.dram_tensor("onesblkF", [P, NGRP], dt.bfloat16, kind="ExternalInput")
    obB_d = nc.dram_tensor("onesblkB", [PB, NGRP], dt.bfloat16, kind="ExternalInput")
    oc_d = nc.dram_tensor("onesbc", [NGRP, P], dt.bfloat16, kind="ExternalInput")

    qF_o = nc.dram_tensor("qF", [P, FD], dt.bfloat16, kind="ExternalOutput")
    qB_o = nc.dram_tensor("qB", [P, FD], dt.bfloat16, kind="ExternalOutput")
    rdF_o = nc.dram_tensor("rdF", [NGRP, max(1, n_ev) * FD], dt.bfloat16,
                           kind="ExternalOutput")
    rdB_o = nc.dram_tensor("rdB", [NGRP, max(1, n_ev) * FD], dt.bfloat16,
                           kind="ExternalOutput")

    with tile.TileContext(nc) as tc:
        with (
            tc.tile_pool(name="singles", bufs=1) as singles,
            tc.tile_pool(name="efpool", bufs=2) as efpool,
            tc.tile_pool(name="small", bufs=2) as small,
            tc.tile_pool(name="ypool", bufs=4) as ypool,
            tc.tile_pool(name="fpool", bufs=4) as fpool,
            tc.tile_pool(name="psF", bufs=3, space="PSUM") as psf_pool,
            tc.tile_pool(name="psB", bufs=3, space="PSUM") as psb_pool,
            tc.tile_pool(name="psE", bufs=1, space="PSUM") as pse_pool,
        ):
            wF_t = singles.tile([P, P], dt.bfloat16, tag="wF", name="wF_t")
            wB_t = singles.tile([PB, PB], dt.bfloat16, tag="wB", name="wB_t")
            obF_t = singles.tile([P, NGRP], dt.bfloat16, tag="obF", name="obF_t")
            obB_t = singles.tile([PB, NGRP], dt.bfloat16, tag="obB", name="obB_t")
            oc_t = singles.tile([NGRP, P], dt.bfloat16, tag="oc", name="oc_t")
            for tl, dr in ((wF_t, wF_d), (wB_t, wB_d), (obF_t, obF_d),
                           (obB_t, obB_d), (oc_t, oc_d)):
                nc.sync.dma_start(out=tl, in_=dr.ap())

            rbF = singles.tile([NGRP, max(1, n_ev) * FD], dt.bfloat16,
                               tag="rbF", name="rbF")
            rbB = singles.tile([NGRP, max(1, n_ev) * FD], dt.bfloat16,
                               tag="rbB", name="rbB")
            f_cur = fpool.tile([P, FD], dt.bfloat16, tag="f", name="f_0")
            nc.sync.dma_start(out=f_cur, in_=qiF_d.ap())

            y_cur = ypool.tile([PB, FD], dt.bfloat16, tag="y", name="y_0")
            nc.sync.dma_start(out=y_cur, in_=y0_d.ap())

            chF = [None, None]
            chB = [None, None]
            pendF, pendB = {}, {}

            def event(i, cur, ob_t, rbuf, pend, is_b):
                # phase 1: colsum + reciprocal now; the broadcast matmul is
                # deferred to the apply step so its semaphore wait never
                # head-of-line-blocks the main matmul stream on PE.
                e = (i - EV0) // NK
                psc = pse_pool.tile([NGRP, FD], dt.float32, tag="psC",
                                    name=f"psC{int(is_b)}_{i}")
                nc.tensor.matmul(psc, ob_t, cur, start=True, stop=True)
                rf = small.tile([NGRP, FD], dt.float32, tag="rf",
                                name=f"rf{int(is_b)}_{i}")
                nc.vector.reciprocal_approx_fast(out=rf, in_=psc)
                rsb = rbuf[:, e * FD:(e + 1) * FD]
                nc.vector.tensor_copy(rsb, rf)
                if i + LAG < (m - 1 if is_b else m):
                    pend[i + LAG] = rsb

            # small leading chunks so the chains start without waiting on
            # a full 1 MB ef transfer; F on the SP HWDGE ring, B on the ACT
            # ring so the two streams don't serialize on one DMA FIFO.
            bounds = [0]
            for inc in (8, 16, 32, 64):
                if bounds[-1] < m:
                    bounds.append(min(m, bounds[-1] + inc))
            while bounds[-1] < m:
                bounds.append(min(m, bounds[-1] + chunk))
            spans = list(zip(bounds[:-1], bounds[1:]))
            for ch, (c_lo, c_hi) in enumerate(spans):
                cw = c_hi - c_lo
                tF = efpool.tile([P, chunk * FD], dt.bfloat16, tag="efF",
                                 name=f"efF_{ch}")
                nc.sync.dma_start(
                    out=tF[:, 0:cw * FD],
                    in_=efF_d.ap()[:, c_lo * FD:c_hi * FD])
                chF[ch % 2] = tF
                tB = efpool.tile([PB, chunk * FD], dt.bfloat16, tag="efB",
                                 name=f"efB_{ch}")
                nc.scalar.dma_start(
                    out=tB[:, 0:cw * FD],
                    in_=efB_d.ap()[:, c_lo * FD:c_hi * FD])
                chB[ch % 2] = tB

                for i in range(c_lo, c_hi):
                    csl = slice((i - c_lo) * FD, (i - c_lo) * FD + FD)
                    # ---------------- forward chain, step i -----------------
                    curF = f_cur
                    if i >= EV0 and (i - EV0) % NK == 0:
                        event(i, curF, obF_t, rbF, pendF, is_b=False)
                    eslF = chF[ch % 2][:, csl]
                    if i in pendF:
                        rsb = pendF.pop(i)
                        psr = pse_pool.tile([P, FD], dt.float32, tag="psR",
                                            name=f"psRF_{i}")
                        nc.tensor.matmul(psr, oc_t, rsb, start=True, stop=True)
                        efx = small.tile([P, FD], dt.bfloat16, tag="efxF",
                                         name=f"efxF_{i}")
                        nc.vector.tensor_mul(efx, psr, eslF)
                        eslF = efx
                    psf = psf_pool.tile([P, FD], dt.float32, tag="psf",
                                        name=f"psf_{i}")
                    nc.tensor.matmul(psf, wF_t, curF, start=True, stop=True)
                    nxtF = fpool.tile([P, FD], dt.bfloat16, tag="f",
                                      name=f"f_{i + 1}")
                    nc.vector.tensor_mul(nxtF, psf, eslF)
                    f_cur = nxtF

                    # ---------------- backward chain, step i ----------------
                    if i >= EV0 and (i - EV0) % NK == 0:
                        event(i, y_cur, obB_t, rbB, pendB, is_b=True)
                    psb = psb_pool.tile([PB, FD], dt.float32, tag="psb",
                                        name=f"psb_{i}")
                    nc.tensor.matmul(psb, wB_t, y_cur, start=True, stop=True)
                    if i < m - 1:
                        eslB = chB[ch % 2][:, csl]
                        if i in pendB:
                            rsb = pendB.pop(i)
                            psr = pse_pool.tile([P, FD], dt.float32, tag="psR",
                                                name=f"psRB_{i}")
                            nc.tensor.matmul(psr, oc_t, rsb, start=True,
                                             stop=True)
                            efx = small.tile([PB, FD], dt.bfloat16, tag="efxB",
                                             name=f"efxB_{i}")
                            nc.vector.tensor_mul(efx[0:P, :], psr, eslB[0:P, :])
                            nc.vector.tensor_copy(efx[P:PB, :], eslB[P:PB, :])
                            eslB = efx
                        y_nxt = ypool.tile([PB, FD], dt.bfloat16, tag="y",
                                           name=f"y_{i + 1}")
                        nc.vector.tensor_mul(y_nxt, psb, eslB)
                        y_cur = y_nxt
                    else:
                        qB_t = singles.tile([P, FD], dt.bfloat16, tag="qBf",
                                            name="qB_t")
                        nc.vector.tensor_copy(qB_t, psb[0:P, :])

            nc.sync.dma_start(out=qF_o.ap(), in_=f_cur)
            nc.sync.dma_start(out=qB_o.ap(), in_=qB_t)
            nc.sync.dma_start(out=rdF_o.ap(), in_=rbF)
            nc.sync.dma_start(out=rdB_o.ap(), in_=rbB)

    nc.finalize()
    return nc


def _host_prep(feats, transition, lengths):
    """Per-core in_maps plus reconstruction metadata."""
    s_len, b_tot = feats.shape[0], feats.shape[1]
    n_cores = b_tot // BC
    m = s_len // 2
    c_pre = feats.max(axis=2)                                # (S, B)
    Ccum = np.vstack([np.zeros((1, b_tot), np.float64),
                      np.cumsum(c_pre.astype(np.float64), 0)])  # (S+1, B)
    ef = np.exp(feats - c_pre[:, :, None]).astype(BF)        # (S, B, T)

    W = np.exp(transition.astype(np.float64))                # [next, prev]
    lhsF = W.T.astype(BF).astype(np.float32)                 # [prev, next]
    lhsB = W.astype(BF).astype(np.float32)                   # [next, prev]
    eT = np.exp(transition[END].astype(np.float64))          # (T,)
    eTb = eT.astype(BF).astype(np.float32)

    wF = np.zeros((P, P), np.float32)
    wB = np.zeros((PB, PB), np.float32)
    for gi in range(NGRP):
        s32 = slice(gi * 32, (gi + 1) * 32)
        wF[s32, s32] = lhsF
        wB[s32, s32] = lhsB
        wB[P + 1 + gi, s32] = eTb                # marker row g -> inject eT
    wB[P, P:PB] = 1.0                            # const row perpetuates
    obF = np.zeros((P, NGRP), np.float32)
    obB = np.zeros((PB, NGRP), np.float32)
    onesbc = np.zeros((NGRP, P), np.float32)
    for gi in range(NGRP):
        obF[gi * 32:(gi + 1) * 32, gi] = 1.0
        obB[gi * 32:(gi + 1) * 32, gi] = 1.0
        onesbc[gi, gi * 32:(gi + 1) * 32] = 1.0
    obB[P, :] = 1.0                              # colsum += 1 (zero-col guard)

    qinitF = np.zeros((P, FD), np.float32)
    qinitF[START, :] = 1.0
    qinitF[32 + START, :] = 1.0

    in_maps = []
    for core in range(n_cores):
        sl = slice(core * BC, (core + 1) * BC)
        A = ef[:, sl, :]                                     # (S, 128, T)
        # brick: [g*32+tag, t, bi] = A[t, g*FD+bi, tag]
        E = (A.reshape(s_len, NGRP, FD, T).transpose(1, 3, 0, 2)
             .reshape(P, s_len, FD)).astype(np.float32)
        EF = np.ascontiguousarray(E[:, :m, :]).reshape(P, m * FD)
        Lc = lengths[sl].astype(int)                         # (128,)
        mark = np.zeros((NGRP, s_len + 1, FD), np.float32)   # [g, t, bi]
        for gi in range(NGRP):
            for bi in range(FD):
                mark[gi, Lc[gi * FD + bi], bi] = 1.0
        # backward stream col i <- t = s_len-2-i, rows: ef, 1, markers at t
        EB = np.zeros((PB, m, FD), np.float32)
        ts = s_len - 2 - np.arange(m)                        # (m,)
        EB[:P] = E[:, ts, :]
        EB[P] = 1.0
        EB[P + 1] = mark[0, ts, :]
        EB[P + 2] = mark[1, ts, :]
        EB = np.ascontiguousarray(EB).reshape(PB, m * FD)
        # y_0: rows = qinitB * ef_{S-1}, const 1, markers at t = S-1
        y0 = np.zeros((PB, FD), np.float32)
        for gi in range(NGRP):
            live = (Lc[gi * FD:(gi + 1) * FD] == s_len).astype(np.float32)
            y0[gi * 32:(gi + 1) * 32, :] = (
                eTb[:, None] * live[None, :] * E[gi * 32:(gi + 1) * 32,
                                                 s_len - 1, :])
        y0[P] = 1.0
        y0[P + 1] = mark[0, s_len - 1, :]
        y0[P + 2] = mark[1, s_len - 1, :]
        in_maps.append({
            "efF": EF.astype(BF),
            "efB": EB.astype(BF),
            "y0": y0.astype(BF),
            "qinitF": qinitF.astype(BF),
            "wblkF": wF.astype(BF),
            "wblkB": wB.astype(BF),
            "onesblkF": obF.astype(BF),
            "onesblkB": obB.astype(BF),
            "onesbc": onesbc.astype(BF),
        })
    return in_maps, Ccum


def _reconstruct(results, Ccum, transition, lengths, s_len=S):
    m = s_len // 2
    n_cores = len(results)
    n_ev = (m - EV0 - 1) // NK + 1 if m > EV0 else 0
    i_apps = EV0 + NK * np.arange(n_ev) + LAG                # (E,)

    out = np.zeros(n_cores * BC, np.float64)
    for core in range(n_cores):
        res = results[core]
        qF = res["qF"].astype(np.float64).reshape(NGRP, 32, FD)
        qB = res["qB"].astype(np.float64).reshape(NGRP, 32, FD)
        lcF = -np.log(np.maximum(
            res["rdF"].astype(np.float64).reshape(NGRP, n_ev, FD), 1e-300))
        lcB = -np.log(np.maximum(
            res["rdB"].astype(np.float64).reshape(NGRP, n_ev, FD), 1e-300))
        for gi in range(NGRP):
            bs = core * BC + gi * FD + np.arange(FD)
            L = lengths[bs]
            dot = (qF[gi] * qB[gi]).sum(axis=0)              # (FD,)
            base = np.log(np.maximum(dot, 1e-300))
            acc = Ccum[L, bs]
            acc = acc + lcF[gi].sum(axis=0)                  # all F events
            i_inj = (s_len - 1) - L                          # -1 when L==s_len
            incB = (i_apps[:, None] >= i_inj[None, :])       # (E, FD)
            acc = acc + (lcB[gi] * incB).sum(axis=0)
            out[bs] = base + acc
    return out


_CACHED_NC = None
LAST_RESULTS = None         # BassKernelResults of the most recent run


def kernel(feats, mask, transition):
    global _CACHED_NC, LAST_RESULTS
    feats = np.asarray(feats, np.float32)
    mask = np.asarray(mask, np.float32)
    transition = np.asarray(transition, np.float32)
    lengths = mask.sum(axis=0).astype(np.int64)              # (B,)

    in_maps, Ccum = _host_prep(feats, transition, lengths)
    if _CACHED_NC is None:
        _CACHED_NC = build_program()
    trace = bool(int(os.environ.get("CRF_TRACE", "0")))
    if trace:
        try:  # supply the NTFF hook module this image's antenv lacks
            import types
            from trn_agent_boot.trn_boot import _ntff_profile_via_ctypes
            if "antenv.axon_hooks" not in sys.modules:
                mm_ = types.ModuleType("antenv.axon_hooks")
                mm_._HOOK = None
                mm_.set_axon_ntff_profile_hook = lambda h: setattr(mm_, "_HOOK", h)
                mm_.get_axon_ntff_profile_hook = lambda: mm_._HOOK
                sys.modules["antenv.axon_hooks"] = mm_
            sys.modules["antenv.axon_hooks"].set_axon_ntff_profile_hook(
                _ntff_profile_via_ctypes("/opt/axon/libaxon_pjrt.so"))
        except Exception as e:  # profiling degrades, run still works
            print(f"ntff hook registration failed: {e}")
    res = run_bass_kernel_spmd(_CACHED_NC, in_maps, core_ids=list(range(NCORES)),
                               trace=trace)
    LAST_RESULTS = res
    out = _reconstruct(res.results, Ccum, transition, lengths)
    return out.astype(np.float32)


if __name__ == "__main__":
    feats = np.load("/tmp/in_feats.npy")
    mask = np.load("/tmp/in_mask.npy")
    trans = np.load("/tmp/in_transition.npy")
    got = kernel(feats, mask, trans)
    exp = np.load("/tmp/expected.npy")
    rel = np.abs(got - exp) / np.maximum(1.0, np.abs(exp))
    print("max rel:", rel.max(), "mean:", rel.mean())



# revision 14
# speedup vs baseline: 1.6802x; 1.6802x over previous
"""CRF forward (partition function) kernel for Trainium2, 8 NeuronCores.

K=4 rank-1 segment decomposition (exp space), data-parallel over batch.
The 1024-step recurrence splits into 4 segments of Q=256.  Products of
256 random positive matrices contract to rank-1 (Perron), so the middle
segments' transfer operators factor as T ~ f g^T / d with f from a
forward chain and g from a backward chain, both seeded with ones; the
host combines  out = g1.a0/d1 * [g2.f1/d2 * (b3.f2) + (c2.f1)]  in log
space, where a0 is the exact forward over [0,256), b3 the exact backward
over [768,1024) with end-of-sequence injections, and c2 a zero-seeded
injection chain over [512,768) (lengths in [512,1024] land in segs 2-3).

All 7 chains pack into 2 matmul bundles that advance together, one
(matmul -> elementwise ef-mul) pair per step:
  bundle X, 128 partitions: a0 | f1 | f2 | g1   (4 x 32 tags, 128 batch)
  bundle Y,  99 partitions: g2 | c2 | b3 | const row | 2 marker rows
Backward chains' final column multiplies by ef:=1 (host-padded), so every
chain runs exactly Q uniform steps.  Injections ride the marker rows as
in the meet-in-the-middle kernel: const row -> marker rows -> eT wiring
inside the stationary, costing zero extra instructions.

W is prescaled by 1/lambda (mean-field Perron value, host eig) so column
magnitudes random-walk near 1; renorm events every NK=64 steps per
bundle (colsum matmul + reciprocal, applied LAG steps later to the ef
stream) keep bf16/fp32 in range; exact factors are dumped for host
compensation, and the prescale is repaid as +L*log(lambda) per element.

ef chunks stream via both HWDGE rings (rows split across SP and ACT) so
DMA stays ahead of the ~55 GB/s/ring consumption.
"""

import os
import sys

import numpy as np
import ml_dtypes

if "/opt/trn_rl_repo" not in sys.path:
    sys.path.insert(0, "/opt/trn_rl_repo")

import concourse.bass as bass
import concourse.tile as tile
from concourse import bacc, mybir
from concourse.bass_utils import run_bass_kernel_spmd

BF = ml_dtypes.bfloat16
S, B, T = 1024, 1024, 32
START, END = T - 2, T - 1
NCORES = 8
FD = 128                    # batch elems per core (free dim)
Q = S // 4                  # steps per segment chain
NK, EV0, LAG = 64, 4, 6     # renorm cadence / first event / apply lag
EV0Y = EV0 + NK // 2        # Y events staggered half a cadence
N_EV = (Q - EV0 - 1) // NK + 1
N_EVY = (Q - EV0Y - 1) // NK + 1
PX, PY = 128, 99            # bundle partition counts
NBX, NBY = 4, 3             # tag blocks per bundle
b1, b2, b3, b4 = Q, 2 * Q, 3 * Q, S

dt = mybir.dt


def _chunk_bounds():
    bounds = [0]
    for inc in (8, 16, 32, 64):
        if bounds[-1] < Q:
            bounds.append(min(Q, bounds[-1] + inc))
    while bounds[-1] < Q:
        bounds.append(min(Q, bounds[-1] + 128))
    return list(zip(bounds[:-1], bounds[1:]))


def build_program():
    nc = bacc.Bacc("TRN2", target_bir_lowering=False, num_devices=NCORES)

    efX_d = nc.dram_tensor("efX", [PX, Q * FD], dt.bfloat16, kind="ExternalInput")
    efY_d = nc.dram_tensor("efY", [PY, Q * FD], dt.bfloat16, kind="ExternalInput")
    x0_d = nc.dram_tensor("x0", [PX, FD], dt.bfloat16, kind="ExternalInput")
    y0_d = nc.dram_tensor("y0", [PY, FD], dt.bfloat16, kind="ExternalInput")
    wX_d = nc.dram_tensor("wX", [PX, PX], dt.bfloat16, kind="ExternalInput")
    wY_d = nc.dram_tensor("wY", [PY, PY], dt.bfloat16, kind="ExternalInput")
    obX_d = nc.dram_tensor("obX", [PX, NBX], dt.bfloat16, kind="ExternalInput")
    obY_d = nc.dram_tensor("obY", [PY, NBY], dt.bfloat16, kind="ExternalInput")
    ocX_d = nc.dram_tensor("ocX", [NBX, PX], dt.bfloat16, kind="ExternalInput")
    ocY_d = nc.dram_tensor("ocY", [NBY, PY], dt.bfloat16, kind="ExternalInput")

    qX_o = nc.dram_tensor("qX", [PX, FD], dt.bfloat16, kind="ExternalOutput")
    qY_o = nc.dram_tensor("qY", [PY, FD], dt.bfloat16, kind="ExternalOutput")
    rdX_o = nc.dram_tensor("rdX", [NBX, N_EV * FD], dt.bfloat16,
                           kind="ExternalOutput")
    rdY_o = nc.dram_tensor("rdY", [NBY, N_EVY * FD], dt.bfloat16,
                           kind="ExternalOutput")

    spans = _chunk_bounds()
    with tile.TileContext(nc) as tc:
        with (
            tc.tile_pool(name="singles", bufs=1) as singles,
            tc.tile_pool(name="efpool", bufs=1) as efpool,
            tc.tile_pool(name="small", bufs=2) as small,
            tc.tile_pool(name="xpool", bufs=4) as xpool,
            tc.tile_pool(name="ypool", bufs=4) as ypool,
            tc.tile_pool(name="psX", bufs=3, space="PSUM") as psx_pool,
            tc.tile_pool(name="psY", bufs=3, space="PSUM") as psy_pool,
            tc.tile_pool(name="psE", bufs=1, space="PSUM") as pse_pool,
        ):
            wX_t = singles.tile([PX, PX], dt.bfloat16, tag="wX", name="wX_t")
            wY_t = singles.tile([PY, PY], dt.bfloat16, tag="wY", name="wY_t")
            obX_t = singles.tile([PX, NBX], dt.bfloat16, tag="obX", name="obX_t")
            obY_t = singles.tile([PY, NBY], dt.bfloat16, tag="obY", name="obY_t")
            ocX_t = singles.tile([NBX, PX], dt.bfloat16, tag="ocX", name="ocX_t")
            ocY_t = singles.tile([NBY, PY], dt.bfloat16, tag="ocY", name="ocY_t")
            for tl, dr in ((wX_t, wX_d), (wY_t, wY_d), (obX_t, obX_d),
                           (obY_t, obY_d), (ocX_t, ocX_d), (ocY_t, ocY_d)):
                nc.sync.dma_start(out=tl, in_=dr.ap())

            rbX = singles.tile([NBX, N_EV * FD], dt.bfloat16, tag="rbX",
                               name="rbX")
            rbY = singles.tile([NBY, N_EVY * FD], dt.bfloat16, tag="rbY",
                               name="rbY")
            x_cur = xpool.tile([PX, FD], dt.bfloat16, tag="x", name="x_0")
            nc.sync.dma_start(out=x_cur, in_=x0_d.ap())
            y_cur = ypool.tile([PY, FD], dt.bfloat16, tag="y", name="y_0")
            nc.scalar.dma_start(out=y_cur, in_=y0_d.ap())

            # all ef chunk DMAs issued up front; rows split across both
            # HWDGE rings so each ring carries ~half the bytes.
            HX, HY = PX // 2, 50
            chX, chY = {}, {}
            for ch, (lo, hi) in enumerate(spans):
                cw = hi - lo
                tX = efpool.tile([PX, cw * FD], dt.bfloat16, tag=f"efX{ch}",
                                 name=f"efX_{ch}")
                nc.sync.dma_start(
                    out=tX[0:HX, :],
                    in_=efX_d.ap()[0:HX, lo * FD:hi * FD])
                nc.scalar.dma_start(
                    out=tX[HX:PX, :],
                    in_=efX_d.ap()[HX:PX, lo * FD:hi * FD])
                chX[ch] = tX
                tY = efpool.tile([PY, cw * FD], dt.bfloat16, tag=f"efY{ch}",
                                 name=f"efY_{ch}")
                nc.scalar.dma_start(
                    out=tY[0:HY, :],
                    in_=efY_d.ap()[0:HY, lo * FD:hi * FD])
                nc.sync.dma_start(
                    out=tY[HY:PY, :],
                    in_=efY_d.ap()[HY:PY, lo * FD:hi * FD])
                chY[ch] = tY

            pendX, pendY = {}, {}

            def event(i, cur, ob_t, rbuf, pend, nblk, ev0):
                e = (i - ev0) // NK
                psc = pse_pool.tile([NBX, FD], dt.float32, tag="psC",
                                    name=f"psC{nblk}_{i}")[0:nblk, :]
                nc.tensor.matmul(psc, ob_t, cur, start=True, stop=True)
                rf = small.tile([NBX, FD], dt.float32, tag="rf",
                                name=f"rf{nblk}_{i}")[0:nblk, :]
                nc.vector.reciprocal_approx_fast(out=rf, in_=psc)
                rsb = rbuf[:, e * FD:(e + 1) * FD]
                nc.vector.tensor_copy(rsb, rf)
                if i + LAG < Q:
                    pend[i + LAG] = rsb

            for ch, (lo, hi) in enumerate(spans):
                for i in range(lo, hi):
                    csl = slice((i - lo) * FD, (i - lo) * FD + FD)
                    # ---------------- bundle X, step i ----------------
                    if i >= EV0 and (i - EV0) % NK == 0:
                        event(i, x_cur, obX_t, rbX, pendX, NBX, EV0)
                    eslX = chX[ch][:, csl]
                    if i in pendX:
                        rsb = pendX.pop(i)
                        psr = pse_pool.tile([PX, FD], dt.float32, tag="psR",
                                            name=f"psRX_{i}")
                        nc.tensor.matmul(psr, ocX_t, rsb, start=True, stop=True)
                        efx = small.tile([PX, FD], dt.bfloat16, tag="efxX",
                                         name=f"efxX_{i}")
                        nc.vector.tensor_mul(efx, psr, eslX)
                        eslX = efx
                    psx = psx_pool.tile([PX, FD], dt.float32, tag="psx",
                                        name=f"psx_{i}")
                    nc.tensor.matmul(psx, wX_t, x_cur, start=True, stop=True)
                    x_nxt = xpool.tile([PX, FD], dt.bfloat16, tag="x",
                                       name=f"x_{i + 1}")
                    nc.vector.tensor_mul(x_nxt, psx, eslX)
                    x_cur = x_nxt

                    # ---------------- bundle Y, step i ----------------
                    if i >= EV0Y and (i - EV0Y) % NK == 0:
                        event(i, y_cur, obY_t, rbY, pendY, NBY, EV0Y)
                    eslY = chY[ch][:, csl]
                    if i in pendY:
                        rsb = pendY.pop(i)
                        psr = pse_pool.tile([PX, FD], dt.float32, tag="psR",
                                            name=f"psRY_{i}")[0:PY, :]
                        nc.tensor.matmul(psr, ocY_t, rsb, start=True, stop=True)
                        efy = small.tile([PY, FD], dt.bfloat16, tag="efxY",
                                         name=f"efxY_{i}")
                        nc.vector.tensor_mul(efy[0:96, :], psr[0:96, :],
                                             eslY[0:96, :])
                        nc.vector.tensor_copy(efy[96:PY, :], eslY[96:PY, :])
                        eslY = efy
                    psy = psy_pool.tile([PY, FD], dt.float32, tag="psy",
                                        name=f"psy_{i}")
                    nc.tensor.matmul(psy, wY_t, y_cur, start=True, stop=True)
                    y_nxt = ypool.tile([PY, FD], dt.bfloat16, tag="y",
                                       name=f"y_{i + 1}")
                    nc.vector.tensor_mul(y_nxt, psy, eslY)
                    y_cur = y_nxt

            nc.sync.dma_start(out=qX_o.ap(), in_=x_cur)
            nc.scalar.dma_start(out=qY_o.ap(), in_=y_cur)
            nc.sync.dma_start(out=rdX_o.ap(), in_=rbX)
            nc.scalar.dma_start(out=rdY_o.ap(), in_=rbY)

    nc.finalize()
    return nc


def _host_prep(feats, transition, lengths):
    """Per-core in_maps plus reconstruction metadata."""
    b_tot = feats.shape[1]
    n_cores = b_tot // FD
    c_pre = feats.max(axis=2)                                # (S, B)
    Ccum = np.vstack([np.zeros((1, b_tot), np.float64),
                      np.cumsum(c_pre.astype(np.float64), 0)])
    ef = np.exp(feats - c_pre[:, :, None]).astype(BF)        # (S, B, T)

    ef_mean = np.exp(feats - c_pre[:, :, None]).mean(axis=(0, 1))
    Wd = np.exp(transition.astype(np.float64))               # [next, prev]
    lam = np.abs(np.linalg.eigvals(ef_mean.astype(np.float64)[:, None]
                                   * Wd)).max()
    log_lam = float(np.log(lam))
    Ws = Wd / lam
    lhsF = Ws.T.astype(BF).astype(np.float32)                # [prev, next]
    lhsB = Ws.astype(BF).astype(np.float32)                  # [next, prev]
    eT = np.exp(transition[END].astype(np.float64))          # (T,) unscaled
    eTb = eT.astype(BF).astype(np.float32)

    wX = np.zeros((PX, PX), np.float32)
    for k in range(NBX):
        sl = slice(32 * k, 32 * k + 32)
        wX[sl, sl] = lhsF if k < 3 else lhsB
    wY = np.zeros((PY, PY), np.float32)
    for k in range(NBY):
        sl = slice(32 * k, 32 * k + 32)
        wY[sl, sl] = lhsB
    wY[96, 96] = 1.0
    wY[96, 97] = 1.0
    wY[96, 98] = 1.0
    wY[97, 32:64] = eTb
    wY[98, 64:96] = eTb
    obX = np.zeros((PX, NBX), np.float32)
    ocX = np.zeros((NBX, PX), np.float32)
    for k in range(NBX):
        obX[32 * k:32 * k + 32, k] = 1.0
        ocX[k, 32 * k:32 * k + 32] = 1.0
    obY = np.zeros((PY, NBY), np.float32)
    ocY = np.zeros((NBY, PY), np.float32)
    for k in range(NBY):
        obY[32 * k:32 * k + 32, k] = 1.0
        ocY[k, 32 * k:32 * k + 32] = 1.0
    obY[96, :] = 1.0                             # colsum += 1 (zero-col guard)

    Lall = lengths.astype(int)
    in_maps = []
    for core in range(n_cores):
        sl = slice(core * FD, (core + 1) * FD)
        A = ef[:, sl, :]                                     # (S, 128, T)
        E = np.ascontiguousarray(A.transpose(0, 2, 1)).astype(np.float32)
        Lc = Lall[sl]                                        # (128,)
        mark = np.zeros((S + 1, FD), np.float32)
        mark[Lc, np.arange(FD)] = 1.0

        EX = np.empty((PX, Q, FD), np.float32)
        EX[0:32] = E[0:b1].transpose(1, 0, 2)
        EX[32:64] = E[b1:b2].transpose(1, 0, 2)
        EX[64:96] = E[b2:b3].transpose(1, 0, 2)
        tsg1 = b2 - 2 - np.arange(Q)                         # 510..255
        EX[96:128] = E[tsg1].transpose(1, 0, 2)
        EX[96:128, Q - 1, :] = 1.0                           # last col ef := 1

        EY = np.empty((PY, Q, FD), np.float32)
        tsg2 = b3 - 2 - np.arange(Q)                         # 766..511
        EY[0:32] = E[tsg2].transpose(1, 0, 2)
        EY[0:32, Q - 1, :] = 1.0
        EY[32:64] = EY[0:32]
        tsg3 = b4 - 2 - np.arange(Q)                         # 1022..767
        EY[64:96] = E[tsg3].transpose(1, 0, 2)
        EY[64:96, Q - 1, :] = 1.0
        EY[96] = 1.0
        EY[97] = mark[tsg2]                                  # t=511 col: L==511
        EY[98] = np.where((tsg3 >= b3)[:, None], mark[tsg3], 0.0)

        x0 = np.zeros((PX, FD), np.float32)
        x0[START, :] = 1.0                                   # a0: one-hot
        x0[32:96] = 1.0                                      # f1, f2: ones
        x0[96:128] = E[b2 - 1]                               # g1: ones*ef[511]

        y0 = np.zeros((PY, FD), np.float32)
        y0[0:32] = E[b3 - 1]                                 # g2: ones*ef[767]
        y0[64:96] = eTb[:, None] * mark[b4][None, :] * E[b4 - 1]
        y0[96] = 1.0
        y0[97] = mark[b3 - 1]                                # 1[L==767]
        y0[98] = mark[b4 - 1]                                # 1[L==1023]

        in_maps.append({
            "efX": np.ascontiguousarray(EX).reshape(PX, Q * FD).astype(BF),
            "efY": np.ascontiguousarray(EY).reshape(PY, Q * FD).astype(BF),
            "x0": x0.astype(BF),
            "y0": y0.astype(BF),
            "wX": wX.astype(BF),
            "wY": wY.astype(BF),
            "obX": obX.astype(BF),
            "obY": obY.astype(BF),
            "ocX": ocX.astype(BF),
            "ocY": ocY.astype(BF),
        })
    return in_maps, Ccum, log_lam


def _reconstruct(results, Ccum, lengths, log_lam):
    n_cores = len(results)
    i_appsY = EV0Y + NK * np.arange(N_EVY) + LAG             # (EY,)
    out = np.zeros(n_cores * FD, np.float64)
    for core in range(n_cores):
        res = results[core]
        Xf = res["qX"].astype(np.float64)                    # (128, 128)
        Yf = res["qY"].astype(np.float64)                    # (99, 128)
        lcX = -np.log(np.maximum(
            res["rdX"].astype(np.float64).reshape(NBX, N_EV, FD), 1e-300))
        lcY = -np.log(np.maximum(
            res["rdY"].astype(np.float64).reshape(NBY, N_EVY, FD), 1e-300))
        bs = core * FD + np.arange(FD)
        L = lengths[bs].astype(int)

        a0, f1, f2, g1 = Xf[0:32], Xf[32:64], Xf[64:96], Xf[96:128]
        g2, c2, b3v = Yf[0:32], Yf[32:64], Yf[64:96]

        acc_a0 = Ccum[b1, bs] - Ccum[0, bs] + lcX[0].sum(0)
        acc_f1 = Ccum[b2, bs] - Ccum[b1, bs] + lcX[1].sum(0)
        acc_f2 = Ccum[b3, bs] - Ccum[b2, bs] + lcX[2].sum(0)
        acc_g1 = Ccum[b2, bs] - Ccum[b1, bs] + lcX[3].sum(0)
        acc_g2 = Ccum[b3, bs] - Ccum[b2, bs] + lcY[0].sum(0)
        incC = (i_appsY[:, None] >= ((b3 - 1) - L)[None, :])
        acc_c2 = (Ccum[np.minimum(L, b3), bs] - Ccum[b2, bs]) \
            + (lcY[1] * incC).sum(0)
        incB = (i_appsY[:, None] >= ((b4 - 1) - L)[None, :])
        acc_b3 = (Ccum[np.minimum(L, b4), bs] - Ccum[b3, bs]) \
            + (lcY[2] * incB).sum(0)

        def logdot(x, ax, y, ay):
            d = (x * y).sum(0)
            o = np.full(d.shape, -np.inf)
            nz = d > 0
            o[nz] = np.log(d[nz]) + ax[nz] + ay[nz]
            return o

        ld1 = np.log(np.maximum(g1.sum(0), 1e-300)) + acc_g1
        ld2 = np.log(np.maximum(g2.sum(0), 1e-300)) + acc_g2
        lg1 = logdot(g1, acc_g1, a0, acc_a0) - ld1
        lg2 = logdot(g2, acc_g2, f1, acc_f1) - ld2
        term1 = lg1 + lg2 + logdot(b3v, acc_b3, f2, acc_f2)
        term2 = lg1 + logdot(c2, acc_c2, f1, acc_f1)
        out[bs] = np.logaddexp(term1, term2) + L * log_lam
    return out


_CACHED_NC = None
LAST_RESULTS = None         # BassKernelResults of the most recent run


def kernel(feats, mask, transition):
    global _CACHED_NC, LAST_RESULTS
    feats = np.asarray(feats, np.float32)
    mask = np.asarray(mask, np.float32)
    transition = np.asarray(transition, np.float32)
    lengths = mask.sum(axis=0).astype(np.int64)              # (B,)

    in_maps, Ccum, log_lam = _host_prep(feats, transition, lengths)
    if _CACHED_NC is None:
        _CACHED_NC = build_program()
    trace = bool(int(os.environ.get("CRF_TRACE", "0")))
    if trace:
        try:  # supply the NTFF hook module this image's antenv lacks
            import types
            from trn_agent_boot.trn_boot import _ntff_profile_via_ctypes
            if "antenv.axon_hooks" not in sys.modules:
                mm_ = types.ModuleType("antenv.axon_hooks")
                mm_._HOOK = None
                mm_.set_axon_ntff_profile_hook = lambda h: setattr(mm_, "_HOOK", h)
                mm_.get_axon_ntff_profile_hook = lambda: mm_._HOOK
                sys.modules["antenv.axon_hooks"] = mm_
            sys.modules["antenv.axon_hooks"].set_axon_ntff_profile_hook(
                _ntff_profile_via_ctypes("/opt/axon/libaxon_pjrt.so"))
        except Exception as e:  # profiling degrades, run still works
            print(f"ntff hook registration failed: {e}")
    res = run_bass_kernel_spmd(_CACHED_NC, in_maps, core_ids=list(range(NCORES)),
                               trace=trace)
    LAST_RESULTS = res
    out = _reconstruct(res.results, Ccum, lengths, log_lam)
    return out.astype(np.float32)


if __name__ == "__main__":
    feats = np.load("/tmp/in_feats.npy")
    mask = np.load("/tmp/in_mask.npy")
    trans = np.load("/tmp/in_transition.npy")
    got = kernel(feats, mask, trans)
    exp = np.load("/tmp/expected.npy")
    rel = np.abs(got - exp) / np.maximum(1.0, np.abs(exp))
    print("max rel:", rel.max(), "mean:", rel.mean())


# revision 21
# speedup vs baseline: 2.6165x; 1.5573x over previous
"""CRF forward (partition function) kernel for Trainium2, 8 NeuronCores.

K=4 rank-1 segment decomposition (exp space), data-parallel over batch.
The 1024-step recurrence splits into 4 segments of Q=256.  Products of
256 random positive matrices contract to rank-1 (Perron), so the middle
segments' transfer operators factor as T ~ f g^T / d with f from a
forward chain and g from a backward chain, both seeded with ones; the
host combines  out = g1.a0/d1 * [g2.f1/d2 * (b3.f2) + (c2.f1)]  in log
space, where a0 is the exact forward over [0,256), b3 the exact backward
over [768,1024) with end-of-sequence injections, and c2 a zero-seeded
injection chain over [512,768) (lengths in [512,1024] land in segs 2-3).

All 7 chains pack into 2 matmul bundles that advance together, one
(matmul -> elementwise ef-mul) pair per step:
  bundle X, 128 partitions: a0 | f1 | f2 | g1   (4 x 32 tags, 128 batch)
  bundle Y,  99 partitions: g2 | c2 | b3 | const row | 2 marker rows
Backward chains' final column multiplies by ef:=1 (host-padded), so every
chain runs exactly Q uniform steps.  Injections ride the marker rows as
in the meet-in-the-middle kernel: const row -> marker rows -> eT wiring
inside the stationary, costing zero extra instructions.

W is prescaled by 1/lambda (mean-field Perron value, host eig) so column
magnitudes random-walk near 1; renorm events every NK=64 steps per
bundle (colsum matmul + reciprocal, applied LAG steps later to the ef
stream) keep bf16/fp32 in range; exact factors are dumped for host
compensation, and the prescale is repaid as +L*log(lambda) per element.

ef chunks stream via both HWDGE rings (rows split across SP and ACT) so
DMA stays ahead of the ~55 GB/s/ring consumption.
"""

import os
import sys

import numpy as np
import ml_dtypes

if "/opt/trn_rl_repo" not in sys.path:
    sys.path.insert(0, "/opt/trn_rl_repo")

import concourse.bass as bass
import concourse.tile as tile
from concourse import bacc, mybir
from concourse.bass_utils import run_bass_kernel_spmd

BF = ml_dtypes.bfloat16
F8 = ml_dtypes.float8_e4m3
S, B, T = 1024, 1024, 32
START, END = T - 2, T - 1
NCORES = 8
FD = 128                    # batch elems per core (free dim)
Q = S // 4                  # steps per segment chain
NK, EV0, LAG = 64, 4, 6     # renorm cadence / first event / apply lag
EV0Y = EV0 + NK // 2        # Y events staggered half a cadence
N_EV = (Q - EV0 - 1) // NK + 1
N_EVY = (Q - EV0Y - 1) // NK + 1
PX, PY = 128, 99            # bundle partition counts
NBX, NBY = 4, 3             # tag blocks per bundle
b1, b2, b3, b4 = Q, 2 * Q, 3 * Q, S

dt = mybir.dt


def _chunk_bounds():
    bounds = [0]
    for inc in (8, 16, 32):
        if bounds[-1] < Q:
            bounds.append(min(Q, bounds[-1] + inc))
    while bounds[-1] < Q:
        bounds.append(min(Q, bounds[-1] + 64))
    return list(zip(bounds[:-1], bounds[1:]))


def build_program():
    nc = bacc.Bacc("TRN2", target_bir_lowering=False, num_devices=NCORES)

    efX_d = nc.dram_tensor("efX", [PX, Q * FD], dt.float8e4, kind="ExternalInput")
    efY_d = nc.dram_tensor("efY", [PY, Q * FD], dt.float8e4, kind="ExternalInput")
    x0_d = nc.dram_tensor("x0", [PX, FD], dt.bfloat16, kind="ExternalInput")
    y0_d = nc.dram_tensor("y0", [PY, FD], dt.bfloat16, kind="ExternalInput")
    wX_d = nc.dram_tensor("wX", [PX, PX], dt.bfloat16, kind="ExternalInput")
    wY_d = nc.dram_tensor("wY", [PY, PY], dt.bfloat16, kind="ExternalInput")
    obX_d = nc.dram_tensor("obX", [PX, NBX], dt.bfloat16, kind="ExternalInput")
    obY_d = nc.dram_tensor("obY", [PY, NBY], dt.bfloat16, kind="ExternalInput")
    ocX_d = nc.dram_tensor("ocX", [NBX, PX], dt.bfloat16, kind="ExternalInput")
    ocY_d = nc.dram_tensor("ocY", [NBY, PY], dt.bfloat16, kind="ExternalInput")

    qX_o = nc.dram_tensor("qX", [PX, FD], dt.bfloat16, kind="ExternalOutput")
    qY_o = nc.dram_tensor("qY", [PY, FD], dt.bfloat16, kind="ExternalOutput")
    rdX_o = nc.dram_tensor("rdX", [NBX, N_EV * FD], dt.bfloat16,
                           kind="ExternalOutput")
    rdY_o = nc.dram_tensor("rdY", [NBY, N_EVY * FD], dt.bfloat16,
                           kind="ExternalOutput")

    spans = _chunk_bounds()
    with tile.TileContext(nc) as tc:
        with (
            tc.tile_pool(name="singles", bufs=1) as singles,
            tc.tile_pool(name="efpool", bufs=1) as efpool,
            tc.tile_pool(name="small", bufs=2) as small,
            tc.tile_pool(name="xpool", bufs=4) as xpool,
            tc.tile_pool(name="ypool", bufs=4) as ypool,
            tc.tile_pool(name="psX", bufs=3, space="PSUM") as psx_pool,
            tc.tile_pool(name="psY", bufs=3, space="PSUM") as psy_pool,
            tc.tile_pool(name="psE", bufs=1, space="PSUM") as pse_pool,
        ):
            wX_t = singles.tile([PX, PX], dt.bfloat16, tag="wX", name="wX_t")
            wY_t = singles.tile([PY, PY], dt.bfloat16, tag="wY", name="wY_t")
            obX_t = singles.tile([PX, NBX], dt.bfloat16, tag="obX", name="obX_t")
            obY_t = singles.tile([PY, NBY], dt.bfloat16, tag="obY", name="obY_t")
            ocX_t = singles.tile([NBX, PX], dt.bfloat16, tag="ocX", name="ocX_t")
            ocY_t = singles.tile([NBY, PY], dt.bfloat16, tag="ocY", name="ocY_t")
            for tl, dr in ((wX_t, wX_d), (obX_t, obX_d), (ocX_t, ocX_d)):
                nc.sync.dma_start(out=tl, in_=dr.ap())
            for tl, dr in ((wY_t, wY_d), (obY_t, obY_d), (ocY_t, ocY_d)):
                nc.scalar.dma_start(out=tl, in_=dr.ap())

            rbX = singles.tile([NBX, N_EV * FD], dt.bfloat16, tag="rbX",
                               name="rbX")
            rbY = singles.tile([NBY, N_EVY * FD], dt.bfloat16, tag="rbY",
                               name="rbY")
            x_cur = xpool.tile([PX, FD], dt.bfloat16, tag="x", name="x_0")
            nc.sync.dma_start(out=x_cur, in_=x0_d.ap())
            y_cur = ypool.tile([PY, FD], dt.bfloat16, tag="y", name="y_0")
            nc.scalar.dma_start(out=y_cur, in_=y0_d.ap())

            # all ef chunk DMAs issued up front; rows split across both
            # HWDGE rings so each ring carries ~half the bytes.
            HX, HY = PX // 2, 50
            chX, chY = {}, {}
            for ch, (lo, hi) in enumerate(spans):
                cw = hi - lo
                tX = efpool.tile([PX, cw * FD], dt.float8e4, tag=f"efX{ch}",
                                 name=f"efX_{ch}")
                nc.sync.dma_start(
                    out=tX[0:HX, :],
                    in_=efX_d.ap()[0:HX, lo * FD:hi * FD])
                nc.scalar.dma_start(
                    out=tX[HX:PX, :],
                    in_=efX_d.ap()[HX:PX, lo * FD:hi * FD])
                chX[ch] = tX
                tY = efpool.tile([PY, cw * FD], dt.float8e4, tag=f"efY{ch}",
                                 name=f"efY_{ch}")
                nc.scalar.dma_start(
                    out=tY[0:HY, :],
                    in_=efY_d.ap()[0:HY, lo * FD:hi * FD])
                nc.sync.dma_start(
                    out=tY[HY:PY, :],
                    in_=efY_d.ap()[HY:PY, lo * FD:hi * FD])
                chY[ch] = tY

            pendX, pendY = {}, {}

            def event(i, cur, ob_t, rbuf, pend, nblk, ev0):
                e = (i - ev0) // NK
                psc = pse_pool.tile([NBX, FD], dt.float32, tag="psC",
                                    name=f"psC{nblk}_{i}")[0:nblk, :]
                nc.tensor.matmul(psc, ob_t, cur, start=True, stop=True)
                rf = small.tile([NBX, FD], dt.float32, tag="rf",
                                name=f"rf{nblk}_{i}")[0:nblk, :]
                nc.vector.reciprocal_approx_fast(out=rf, in_=psc)
                rsb = rbuf[:, e * FD:(e + 1) * FD]
                nc.vector.tensor_copy(rsb, rf)
                if i + LAG < Q:
                    pend[i + LAG] = rsb

            for ch, (lo, hi) in enumerate(spans):
                for i in range(lo, hi):
                    csl = slice((i - lo) * FD, (i - lo) * FD + FD)
                    # ---------------- bundle X, step i ----------------
                    if i >= EV0 and (i - EV0) % NK == 0:
                        event(i, x_cur, obX_t, rbX, pendX, NBX, EV0)
                    eslX = chX[ch][:, csl]
                    if i in pendX:
                        rsb = pendX.pop(i)
                        psr = pse_pool.tile([PX, FD], dt.float32, tag="psR",
                                            name=f"psRX_{i}")
                        nc.tensor.matmul(psr, ocX_t, rsb, start=True, stop=True)
                        efx = small.tile([PX, FD], dt.bfloat16, tag="efxX",
                                         name=f"efxX_{i}")
                        nc.vector.tensor_mul(efx, psr, eslX)
                        eslX = efx
                    psx = psx_pool.tile([PX, FD], dt.float32, tag="psx",
                                        name=f"psx_{i}")
                    nc.tensor.matmul(psx, wX_t, x_cur, start=True, stop=True)
                    x_nxt = xpool.tile([PX, FD], dt.bfloat16, tag="x",
                                       name=f"x_{i + 1}")
                    nc.vector.tensor_mul(x_nxt, psx, eslX)
                    x_cur = x_nxt

                    # ---------------- bundle Y, step i ----------------
                    if i >= EV0Y and (i - EV0Y) % NK == 0:
                        event(i, y_cur, obY_t, rbY, pendY, NBY, EV0Y)
                    eslY = chY[ch][:, csl]
                    if i in pendY:
                        rsb = pendY.pop(i)
                        psr = pse_pool.tile([PX, FD], dt.float32, tag="psR",
                                            name=f"psRY_{i}")[0:PY, :]
                        nc.tensor.matmul(psr, ocY_t, rsb, start=True, stop=True)
                        efy = small.tile([PY, FD], dt.bfloat16, tag="efxY",
                                         name=f"efxY_{i}")
                        nc.vector.tensor_mul(efy[0:96, :], psr[0:96, :],
                                             eslY[0:96, :])
                        nc.vector.tensor_copy(efy[96:PY, :], eslY[96:PY, :])
                        eslY = efy
                    psy = psy_pool.tile([PY, FD], dt.float32, tag="psy",
                                        name=f"psy_{i}")
                    nc.tensor.matmul(psy, wY_t, y_cur, start=True, stop=True)
                    y_nxt = ypool.tile([PY, FD], dt.bfloat16, tag="y",
                                       name=f"y_{i + 1}")
                    nc.vector.tensor_mul(y_nxt, psy, eslY)
                    y_cur = y_nxt

            nc.sync.dma_start(out=qX_o.ap(), in_=x_cur)
            nc.scalar.dma_start(out=qY_o.ap(), in_=y_cur)
            nc.sync.dma_start(out=rdX_o.ap(), in_=rbX)
            nc.scalar.dma_start(out=rdY_o.ap(), in_=rbY)

    nc.finalize()
    return nc


def _host_prep(feats, transition, lengths):
    """Per-core in_maps plus reconstruction metadata."""
    b_tot = feats.shape[1]
    n_cores = b_tot // FD
    c_pre = feats.max(axis=2)                                # (S, B)
    Ccum = np.vstack([np.zeros((1, b_tot), np.float64),
                      np.cumsum(c_pre.astype(np.float64), 0)])
    ef = np.exp(feats - c_pre[:, :, None]).astype(BF)        # (S, B, T)

    ef_mean = np.exp(feats - c_pre[:, :, None]).mean(axis=(0, 1))
    Wd = np.exp(transition.astype(np.float64))               # [next, prev]
    lam = np.abs(np.linalg.eigvals(ef_mean.astype(np.float64)[:, None]
                                   * Wd)).max()
    log_lam = float(np.log(lam))
    Ws = Wd / lam
    lhsF = Ws.T.astype(BF).astype(np.float32)                # [prev, next]
    lhsB = Ws.astype(BF).astype(np.float32)                  # [next, prev]
    eT = np.exp(transition[END].astype(np.float64))          # (T,) unscaled
    eTb = eT.astype(BF).astype(np.float32)

    wX = np.zeros((PX, PX), np.float32)
    for k in range(NBX):
        sl = slice(32 * k, 32 * k + 32)
        wX[sl, sl] = lhsF if k < 3 else lhsB
    wY = np.zeros((PY, PY), np.float32)
    for k in range(NBY):
        sl = slice(32 * k, 32 * k + 32)
        wY[sl, sl] = lhsB
    wY[96, 96] = 1.0
    wY[96, 97] = 1.0
    wY[96, 98] = 1.0
    wY[97, 32:64] = eTb
    wY[98, 64:96] = eTb
    obX = np.zeros((PX, NBX), np.float32)
    ocX = np.zeros((NBX, PX), np.float32)
    for k in range(NBX):
        obX[32 * k:32 * k + 32, k] = 1.0
        ocX[k, 32 * k:32 * k + 32] = 1.0
    obY = np.zeros((PY, NBY), np.float32)
    ocY = np.zeros((NBY, PY), np.float32)
    for k in range(NBY):
        obY[32 * k:32 * k + 32, k] = 1.0
        ocY[k, 32 * k:32 * k + 32] = 1.0
    obY[96, :] = 1.0                             # colsum += 1 (zero-col guard)

    Lall = lengths.astype(int)
    in_maps = []
    for core in range(n_cores):
        sl = slice(core * FD, (core + 1) * FD)
        A = ef[:, sl, :]                                     # (S, 128, T)
        E = np.ascontiguousarray(A.transpose(0, 2, 1)).astype(np.float32)
        Lc = Lall[sl]                                        # (128,)
        mark = np.zeros((S + 1, FD), np.float32)
        mark[Lc, np.arange(FD)] = 1.0

        EX = np.empty((PX, Q, FD), np.float32)
        EX[0:32] = E[0:b1].transpose(1, 0, 2)
        EX[32:64] = E[b1:b2].transpose(1, 0, 2)
        EX[64:96] = E[b2:b3].transpose(1, 0, 2)
        tsg1 = b2 - 2 - np.arange(Q)                         # 510..255
        EX[96:128] = E[tsg1].transpose(1, 0, 2)
        EX[96:128, Q - 1, :] = 1.0                           # last col ef := 1

        EY = np.empty((PY, Q, FD), np.float32)
        tsg2 = b3 - 2 - np.arange(Q)                         # 766..511
        EY[0:32] = E[tsg2].transpose(1, 0, 2)
        EY[0:32, Q - 1, :] = 1.0
        EY[32:64] = EY[0:32]
        tsg3 = b4 - 2 - np.arange(Q)                         # 1022..767
        EY[64:96] = E[tsg3].transpose(1, 0, 2)
        EY[64:96, Q - 1, :] = 1.0
        EY[96] = 1.0
        EY[97] = mark[tsg2]                                  # t=511 col: L==511
        EY[98] = np.where((tsg3 >= b3)[:, None], mark[tsg3], 0.0)

        x0 = np.zeros((PX, FD), np.float32)
        x0[START, :] = 1.0                                   # a0: one-hot
        x0[32:96] = 1.0                                      # f1, f2: ones
        x0[96:128] = E[b2 - 1]                               # g1: ones*ef[511]

        y0 = np.zeros((PY, FD), np.float32)
        y0[0:32] = E[b3 - 1]                                 # g2: ones*ef[767]
        y0[64:96] = eTb[:, None] * mark[b4][None, :] * E[b4 - 1]
        y0[96] = 1.0
        y0[97] = mark[b3 - 1]                                # 1[L==767]
        y0[98] = mark[b4 - 1]                                # 1[L==1023]

        in_maps.append({
            "efX": np.ascontiguousarray(EX).reshape(PX, Q * FD).astype(F8),
            "efY": np.ascontiguousarray(EY).reshape(PY, Q * FD).astype(F8),
            "x0": x0.astype(BF),
            "y0": y0.astype(BF),
            "wX": wX.astype(BF),
            "wY": wY.astype(BF),
            "obX": obX.astype(BF),
            "obY": obY.astype(BF),
            "ocX": ocX.astype(BF),
            "ocY": ocY.astype(BF),
        })
    return in_maps, Ccum, log_lam


def _reconstruct(results, Ccum, lengths, log_lam):
    n_cores = len(results)
    i_appsY = EV0Y + NK * np.arange(N_EVY) + LAG             # (EY,)
    out = np.zeros(n_cores * FD, np.float64)
    for core in range(n_cores):
        res = results[core]
        Xf = res["qX"].astype(np.float64)                    # (128, 128)
        Yf = res["qY"].astype(np.float64)                    # (99, 128)
        lcX = -np.log(np.maximum(
            res["rdX"].astype(np.float64).reshape(NBX, N_EV, FD), 1e-300))
        lcY = -np.log(np.maximum(
            res["rdY"].astype(np.float64).reshape(NBY, N_EVY, FD), 1e-300))
        bs = core * FD + np.arange(FD)
        L = lengths[bs].astype(int)

        a0, f1, f2, g1 = Xf[0:32], Xf[32:64], Xf[64:96], Xf[96:128]
        g2, c2, b3v = Yf[0:32], Yf[32:64], Yf[64:96]

        acc_a0 = Ccum[b1, bs] - Ccum[0, bs] + lcX[0].sum(0)
        acc_f1 = Ccum[b2, bs] - Ccum[b1, bs] + lcX[1].sum(0)
        acc_f2 = Ccum[b3, bs] - Ccum[b2, bs] + lcX[2].sum(0)
        acc_g1 = Ccum[b2, bs] - Ccum[b1, bs] + lcX[3].sum(0)
        acc_g2 = Ccum[b3, bs] - Ccum[b2, bs] + lcY[0].sum(0)
        incC = (i_appsY[:, None] >= ((b3 - 1) - L)[None, :])
        acc_c2 = (Ccum[np.minimum(L, b3), bs] - Ccum[b2, bs]) \
            + (lcY[1] * incC).sum(0)
        incB = (i_appsY[:, None] >= ((b4 - 1) - L)[None, :])
        acc_b3 = (Ccum[np.minimum(L, b4), bs] - Ccum[b3, bs]) \
            + (lcY[2] * incB).sum(0)

        def logdot(x, ax, y, ay):
            d = (x * y).sum(0)
            o = np.full(d.shape, -np.inf)
            nz = d > 0
            o[nz] = np.log(d[nz]) + ax[nz] + ay[nz]
            return o

        ld1 = np.log(np.maximum(g1.sum(0), 1e-300)) + acc_g1
        ld2 = np.log(np.maximum(g2.sum(0), 1e-300)) + acc_g2
        lg1 = logdot(g1, acc_g1, a0, acc_a0) - ld1
        lg2 = logdot(g2, acc_g2, f1, acc_f1) - ld2
        term1 = lg1 + lg2 + logdot(b3v, acc_b3, f2, acc_f2)
        term2 = lg1 + logdot(c2, acc_c2, f1, acc_f1)
        out[bs] = np.logaddexp(term1, term2) + L * log_lam
    return out


_CACHED_NC = None
LAST_RESULTS = None         # BassKernelResults of the most recent run


def kernel(feats, mask, transition):
    global _CACHED_NC, LAST_RESULTS
    feats = np.asarray(feats, np.float32)
    mask = np.asarray(mask, np.float32)
    transition = np.asarray(transition, np.float32)
    lengths = mask.sum(axis=0).astype(np.int64)              # (B,)

    in_maps, Ccum, log_lam = _host_prep(feats, transition, lengths)
    if _CACHED_NC is None:
        _CACHED_NC = build_program()
    trace = bool(int(os.environ.get("CRF_TRACE", "0")))
    if trace:
        try:  # supply the NTFF hook module this image's antenv lacks
            import types
            from trn_agent_boot.trn_boot import _ntff_profile_via_ctypes
            if "antenv.axon_hooks" not in sys.modules:
                mm_ = types.ModuleType("antenv.axon_hooks")
                mm_._HOOK = None
                mm_.set_axon_ntff_profile_hook = lambda h: setattr(mm_, "_HOOK", h)
                mm_.get_axon_ntff_profile_hook = lambda: mm_._HOOK
                sys.modules["antenv.axon_hooks"] = mm_
            sys.modules["antenv.axon_hooks"].set_axon_ntff_profile_hook(
                _ntff_profile_via_ctypes("/opt/axon/libaxon_pjrt.so"))
        except Exception as e:  # profiling degrades, run still works
            print(f"ntff hook registration failed: {e}")
    res = run_bass_kernel_spmd(_CACHED_NC, in_maps, core_ids=list(range(NCORES)),
                               trace=trace)
    LAST_RESULTS = res
    out = _reconstruct(res.results, Ccum, lengths, log_lam)
    return out.astype(np.float32)


if __name__ == "__main__":
    feats = np.load("/tmp/in_feats.npy")
    mask = np.load("/tmp/in_mask.npy")
    trans = np.load("/tmp/in_transition.npy")
    got = kernel(feats, mask, trans)
    exp = np.load("/tmp/expected.npy")
    rel = np.abs(got - exp) / np.maximum(1.0, np.abs(exp))
    print("max rel:", rel.max(), "mean:", rel.mean())


# revision 25
# speedup vs baseline: 2.6211x; 1.0018x over previous
"""CRF forward kernel for Trainium2, 8 NeuronCores — K=6 rank-1 segments.

The 1024-step recurrence splits into 6 segments (boundaries
0,170,341,512,682,853,1024); products of ~170 random positive matrices
are rank-1 to machine precision (Perron contraction), so middle segments
factor as T ~ f g^T / d from independently seeded forward/backward
chains.  12 chains pack exactly into 3 bundles of 128 partitions
(4 blocks x 32 tags), each advancing one (matmul -> ef-mul) per step:

  A (170 steps): a0 | f3 | g3 | c3   const=g3.END, marker=c3.END
  B (171 steps): f1 | g1 | f2 | g2   no carriers
  C (171 steps): f4 | g4 | c4 | b5   const=g4.END, markers=c4/b5.END

Carrier rows live in the always-zero rows of backward blocks (W's END
column is forbidden).  Injection needs no extra wiring: a marker value
in a backward block's END row propagates through the natural
W[END,:] = exp(trans[END]) row of the scaled stationary (hence the
(L+1)*log lambda repayment).  Const rows self-normalize via a unit
colsum column in the renorm events; dumped factors make every scale
exactly compensable on the host.

Per step: PE runs 3 matmuls (the binding resource, ~800ns/step), DVE
muls bundles A/B, GPSIMD muls bundle C.  ef streams are fp8 (2.6e-4
rel in simulation, vs the 2e-2 gate), chunk-DMA'd up front across both
HWDGE rings.
"""

import os
import sys

import numpy as np
import ml_dtypes

if "/opt/trn_rl_repo" not in sys.path:
    sys.path.insert(0, "/opt/trn_rl_repo")

import concourse.bass as bass
import concourse.tile as tile
from concourse import bacc, mybir
from concourse.bass_utils import run_bass_kernel_spmd

BF = ml_dtypes.bfloat16
F8 = ml_dtypes.float8_e4m3
S, B, T = 1024, 1024, 32
START, END = T - 2, T - 1
NCORES = 8
FD = 128
NK, LAG = 128, 6
BND = [0, 170, 341, 512, 682, 853, 1024]
QA, QB, QC = 170, 171, 171
EV0 = {"A": 4, "B": 25, "C": 46}
NEV = {k: (q - EV0[k] - 1) // NK + 1
       for k, q in (("A", QA), ("B", QB), ("C", QC))}
NCOL = {"A": 5, "B": 4, "C": 5}
BLK = [slice(32 * k, 32 * k + 32) for k in range(4)]

dt = mybir.dt


def _chunk_bounds(q):
    bounds = [0]
    for inc in (8, 16, 32):
        bounds.append(min(q, bounds[-1] + inc))
    while bounds[-1] < q:
        bounds.append(min(q, bounds[-1] + 64))
    return list(zip(bounds[:-1], bounds[1:]))


def build_program():
    nc = bacc.Bacc("TRN2", target_bir_lowering=False, num_devices=NCORES)

    d = {}
    for k, q in (("A", QA), ("B", QB), ("C", QC)):
        d[f"ef{k}"] = nc.dram_tensor(f"ef{k}", [128, q * FD], dt.float8e4,
                                     kind="ExternalInput")
        d[f"s0{k}"] = nc.dram_tensor(f"s0{k}", [128, FD], dt.bfloat16,
                                     kind="ExternalInput")
        d[f"w{k}"] = nc.dram_tensor(f"w{k}", [128, 128], dt.bfloat16,
                                    kind="ExternalInput")
        d[f"ob{k}"] = nc.dram_tensor(f"ob{k}", [128, NCOL[k]], dt.bfloat16,
                                     kind="ExternalInput")
        d[f"oc{k}"] = nc.dram_tensor(f"oc{k}", [NCOL[k], 128], dt.bfloat16,
                                     kind="ExternalInput")
        d[f"q{k}"] = nc.dram_tensor(f"q{k}", [128, FD], dt.bfloat16,
                                    kind="ExternalOutput")
        d[f"rd{k}"] = nc.dram_tensor(f"rd{k}", [NCOL[k], NEV[k] * FD],
                                     dt.bfloat16, kind="ExternalOutput")

    with tile.TileContext(nc) as tc:
        with (
            tc.tile_pool(name="singles", bufs=1) as singles,
            tc.tile_pool(name="efpool", bufs=1) as efpool,
            tc.tile_pool(name="small", bufs=2) as small,
            tc.tile_pool(name="stA", bufs=3) as stA,
            tc.tile_pool(name="stB", bufs=3) as stB,
            tc.tile_pool(name="stC", bufs=3) as stC,
            tc.tile_pool(name="psA", bufs=2, space="PSUM") as psA,
            tc.tile_pool(name="psB", bufs=2, space="PSUM") as psB,
            tc.tile_pool(name="psC", bufs=2, space="PSUM") as psC,
            tc.tile_pool(name="psE", bufs=1, space="PSUM") as psE,
        ):
            t = {}
            ring = {"A": nc.sync, "B": nc.scalar, "C": nc.sync}
            ring2 = {"A": nc.scalar, "B": nc.sync, "C": nc.scalar}
            # first ef chunk ahead of everything so compute starts early
            spans = {k: _chunk_bounds(q)
                     for k, q in (("A", QA), ("B", QB), ("C", QC))}
            chunks = {}
            efpool_tiles = {}
            for k in ("A", "B", "C"):
                lo, hi = spans[k][0]
                cw = hi - lo
                tl = efpool.tile([128, cw * FD], dt.float8e4,
                                 tag=f"ef{k}0", name=f"ef{k}_0")
                ring[k].dma_start(out=tl[0:64, :],
                                  in_=d[f"ef{k}"].ap()[0:64, lo * FD:hi * FD])
                ring2[k].dma_start(out=tl[64:128, :],
                                   in_=d[f"ef{k}"].ap()[64:128,
                                                        lo * FD:hi * FD])
                chunks[(k, 0)] = (tl, lo, hi)
            for k in ("A", "B", "C"):
                t[f"w{k}"] = singles.tile([128, 128], dt.bfloat16,
                                          tag=f"w{k}", name=f"w{k}_t")
                t[f"ob{k}"] = singles.tile([128, NCOL[k]], dt.bfloat16,
                                           tag=f"ob{k}", name=f"ob{k}_t")
                t[f"oc{k}"] = singles.tile([NCOL[k], 128], dt.bfloat16,
                                           tag=f"oc{k}", name=f"oc{k}_t")
                r = ring[k]
                r.dma_start(out=t[f"w{k}"], in_=d[f"w{k}"].ap())
                r.dma_start(out=t[f"ob{k}"], in_=d[f"ob{k}"].ap())
                r.dma_start(out=t[f"oc{k}"], in_=d[f"oc{k}"].ap())
                t[f"rb{k}"] = singles.tile([NCOL[k], NEV[k] * FD],
                                           dt.bfloat16, tag=f"rb{k}",
                                           name=f"rb{k}")

            pools = {"A": stA, "B": stB, "C": stC}
            cur = {}
            for k in ("A", "B", "C"):
                cur[k] = pools[k].tile([128, FD], dt.bfloat16, tag="s",
                                       name=f"s{k}_0")
                ring2[k].dma_start(out=cur[k], in_=d[f"s0{k}"].ap())

            nch = len(spans["A"])
            for ch in range(1, nch):
                for k in ("A", "B", "C"):
                    if ch >= len(spans[k]):
                        continue
                    lo, hi = spans[k][ch]
                    cw = hi - lo
                    tl = efpool.tile([128, cw * FD], dt.float8e4,
                                     tag=f"ef{k}{ch}", name=f"ef{k}_{ch}")
                    ring[k].dma_start(
                        out=tl[0:64, :],
                        in_=d[f"ef{k}"].ap()[0:64, lo * FD:hi * FD])
                    ring2[k].dma_start(
                        out=tl[64:128, :],
                        in_=d[f"ef{k}"].ap()[64:128, lo * FD:hi * FD])
                    chunks[(k, ch)] = (tl, lo, hi)

            pend = {"A": {}, "B": {}, "C": {}}

            def event(k, i, mul_engine):
                e = (i - EV0[k]) // NK
                ncol = NCOL[k]
                psc = psE.tile([5, FD], dt.float32, tag="psC",
                               name=f"psC{k}_{i}")[0:ncol, :]
                nc.tensor.matmul(psc, t[f"ob{k}"], cur[k], start=True,
                                 stop=True)
                rf = small.tile([5, FD], dt.float32, tag="rf",
                                name=f"rf{k}_{i}")[0:ncol, :]
                nc.vector.reciprocal_approx_fast(out=rf, in_=psc)
                rsb = t[f"rb{k}"][:, e * FD:(e + 1) * FD]
                nc.vector.tensor_copy(rsb, rf)
                q = {"A": QA, "B": QB, "C": QC}[k]
                if i + LAG < q:
                    pend[k][i + LAG] = rsb

            def step(k, i, ch, lo, tl, ps_pool, mul_engine):
                if i >= EV0[k] and (i - EV0[k]) % NK == 0:
                    event(k, i, mul_engine)
                csl = slice((i - lo) * FD, (i - lo) * FD + FD)
                esl = tl[:, csl]
                if i in pend[k]:
                    rsb = pend[k].pop(i)
                    psr = psE.tile([128, FD], dt.float32, tag="psR",
                                   name=f"psR{k}_{i}")
                    nc.tensor.matmul(psr, t[f"oc{k}"], rsb, start=True,
                                     stop=True)
                    efx = small.tile([128, FD], dt.bfloat16, tag="efx",
                                     name=f"efx{k}_{i}")
                    nc.vector.tensor_mul(efx, psr, esl)
                    esl = efx
                ps = ps_pool.tile([128, FD], dt.float32, tag="ps",
                                  name=f"ps{k}_{i}")
                nc.tensor.matmul(ps, t[f"w{k}"], cur[k], start=True,
                                 stop=True)
                nxt = pools[k].tile([128, FD], dt.bfloat16, tag="s",
                                    name=f"s{k}_{i + 1}")
                mul_engine.tensor_mul(nxt, ps, esl)
                cur[k] = nxt

            chi = {"A": 0, "B": 0, "C": 0}
            for i in range(QC):
                for k, q, psp, eng in (("A", QA, psA, nc.vector),
                                       ("B", QB, psB, nc.vector),
                                       ("C", QC, psC, nc.vector)):
                    if i >= q:
                        continue
                    if i >= spans[k][chi[k]][1]:
                        chi[k] += 1
                    tl, lo, hi = chunks[(k, chi[k])]
                    step(k, i, chi[k], lo, tl, psp, eng)

            for k in ("A", "B", "C"):
                ring[k].dma_start(out=d[f"q{k}"].ap(), in_=cur[k])
                ring2[k].dma_start(out=d[f"rd{k}"].ap(), in_=t[f"rb{k}"])

    nc.finalize()
    return nc


def _host_prep(feats, transition, lengths):
    b_tot = feats.shape[1]
    n_cores = b_tot // FD
    b0, b1, b2, b3, b4, b5, b6 = BND
    c_pre = feats.max(axis=2)                                # (S, B)
    Ccum = np.vstack([np.zeros((1, b_tot), np.float64),
                      np.cumsum(c_pre.astype(np.float64), 0)])
    efq = np.exp(feats - c_pre[:, :, None]).astype(np.float32)   # (S,B,T)

    ef_mean = efq.mean(axis=(0, 1)).astype(np.float64)
    Wd = np.exp(transition.astype(np.float64))
    lam = np.abs(np.linalg.eigvals(ef_mean[:, None] * Wd)).max()
    log_lam = float(np.log(lam))
    Ws = Wd / lam
    lhsF = Ws.T
    lhsB = Ws
    eT = np.exp(transition[END].astype(np.float64))

    def bundle_w(kinds, const_blk, mark_blks):
        Wm = np.zeros((128, 128))
        for k, kind in enumerate(kinds):
            Wm[BLK[k], BLK[k]] = lhsF if kind == 'F' else lhsB
        if const_blk is not None:
            ce = 32 * const_blk + END
            Wm[ce, BLK[const_blk]] = 0.0
            Wm[ce, ce] = 1.0
            for mb in mark_blks:
                Wm[ce, 32 * mb + END] = 1.0
        return Wm.astype(BF)

    def bundle_oboc(const_blk, mark_blks, guard_cblks, ncol):
        ob = np.zeros((128, ncol), np.float32)
        oc = np.zeros((ncol, 128), np.float32)
        carrier = []
        if const_blk is not None:
            carrier.append(32 * const_blk + END)
            carrier += [32 * mb + END for mb in mark_blks]
        for k in range(4):
            rows = [r for r in range(32 * k, 32 * k + 32) if r not in carrier]
            ob[rows, k] = 1.0
            oc[k, rows] = 1.0
        if const_blk is not None:
            ce = 32 * const_blk + END
            ob[ce, ncol - 1] = 1.0
            oc[ncol - 1, ce] = 1.0
            for mb in mark_blks:
                oc[ncol - 1, 32 * mb + END] = 1.0
            for cb in guard_cblks:
                ob[ce, cb] = 1.0
        return ob.astype(BF), oc.astype(BF)

    wA = bundle_w(['F', 'F', 'B', 'B'], 2, [3])
    wB = bundle_w(['F', 'B', 'F', 'B'], None, [])
    wC = bundle_w(['F', 'B', 'B', 'B'], 1, [2, 3])
    obA, ocA = bundle_oboc(2, [3], [3], 5)
    obB, ocB = bundle_oboc(None, [], [], 4)
    obC, ocC = bundle_oboc(1, [2, 3], [2, 3], 5)

    Lall = lengths.astype(int)
    in_maps = []
    for core in range(n_cores):
        sl = slice(core * FD, (core + 1) * FD)
        E = np.ascontiguousarray(efq[:, sl, :].transpose(0, 2, 1))  # (S,T,FD)
        Lc = Lall[sl]
        mark = np.zeros((S + 1, FD), np.float32)
        mark[Lc, np.arange(FD)] = 1.0

        def fcols(a, b, q):
            return E[a:b].transpose(1, 0, 2)                 # (T, q, FD)

        def bcols(a, b, q, mark_lo=None, mark_hi=None, zero_end=False):
            ts = b - 2 - np.arange(q)
            out = np.stack([E[tt] if tt >= a else np.ones((T, FD), np.float32)
                            for tt in ts], axis=1)
            if zero_end:
                out[END] = 1.0
            if mark_lo is not None:
                out[END] = np.stack(
                    [mark[tt] if mark_lo <= tt <= mark_hi
                     else np.zeros(FD, np.float32) for tt in ts], axis=0)
            return out

        efA = np.concatenate([
            fcols(b0, b1, QA), fcols(b3, b4, QA),
            bcols(b3, b4, QA, zero_end=True),
            bcols(b3, b4, QA, mark_lo=b3, mark_hi=b4 - 2)], axis=0)
        efB = np.concatenate([
            fcols(b1, b2, QB), bcols(b1, b2, QB),
            fcols(b2, b3, QB), bcols(b2, b3, QB)], axis=0)
        efC = np.concatenate([
            fcols(b4, b5, QC), bcols(b4, b5, QC, zero_end=True),
            bcols(b4, b5, QC, mark_lo=b4, mark_hi=b5 - 2),
            bcols(b5, b6, QC, mark_lo=b5, mark_hi=b6 - 2)], axis=0)

        def bseed(b):
            s = E[b - 1].copy()
            s[END] = 0.0
            return s

        A0 = np.zeros((128, FD), np.float32)
        A0[START] = 1.0
        A0[BLK[1]] = 1.0
        A0[BLK[2]] = bseed(b4)
        A0[64 + END] = 1.0
        A0[96 + END] = mark[b4 - 1]

        B0 = np.zeros((128, FD), np.float32)
        B0[BLK[0]] = 1.0
        B0[BLK[1]] = bseed(b2)
        B0[BLK[2]] = 1.0
        B0[BLK[3]] = bseed(b3)

        C0 = np.zeros((128, FD), np.float32)
        C0[BLK[0]] = 1.0
        C0[BLK[1]] = bseed(b5)
        C0[32 + END] = 1.0
        C0[64 + END] = mark[b5 - 1]
        C0[BLK[3]] = (eT / lam)[:, None].astype(np.float32) \
            * mark[b6][None, :] * E[b6 - 1]
        C0[96 + END] = mark[b6 - 1]

        in_maps.append({
            "efA": np.ascontiguousarray(efA).reshape(128, QA * FD).astype(F8),
            "efB": np.ascontiguousarray(efB).reshape(128, QB * FD).astype(F8),
            "efC": np.ascontiguousarray(efC).reshape(128, QC * FD).astype(F8),
            "s0A": A0.astype(BF), "s0B": B0.astype(BF), "s0C": C0.astype(BF),
            "wA": wA, "wB": wB, "wC": wC,
            "obA": obA, "obB": obB, "obC": obC,
            "ocA": ocA, "ocB": ocB, "ocC": ocC,
        })
    return in_maps, Ccum, log_lam


def _reconstruct(results, Ccum, lengths, log_lam):
    n_cores = len(results)
    b0, b1, b2, b3, b4, b5, b6 = BND
    out = np.zeros(n_cores * FD, np.float64)
    for core in range(n_cores):
        res = results[core]
        Af = res["qA"].astype(np.float64)
        Bf = res["qB"].astype(np.float64)
        Cf = res["qC"].astype(np.float64)
        lcA = -np.log(np.maximum(res["rdA"].astype(np.float64)
                                 .reshape(5, NEV["A"], FD), 1e-300))
        lcB = -np.log(np.maximum(res["rdB"].astype(np.float64)
                                 .reshape(4, NEV["B"], FD), 1e-300))
        lcC = -np.log(np.maximum(res["rdC"].astype(np.float64)
                                 .reshape(5, NEV["C"], FD), 1e-300))
        bs = core * FD + np.arange(FD)
        L = lengths[bs].astype(int)

        def blk(Xf, k, zero_end=False):
            v = Xf[BLK[k]].copy()
            if zero_end:
                v[END] = 0.0
            return v

        a0 = blk(Af, 0)
        f3 = blk(Af, 1)
        g3 = blk(Af, 2, True)
        c3 = blk(Af, 3, True)
        f1 = blk(Bf, 0)
        g1 = blk(Bf, 1)
        f2 = blk(Bf, 2)
        g2 = blk(Bf, 3)
        f4 = blk(Cf, 0)
        g4 = blk(Cf, 1, True)
        c4 = blk(Cf, 2, True)
        b5v = blk(Cf, 3, True)

        def CC(a, b):
            return Ccum[b, bs] - Ccum[a, bs]

        acc_a0 = CC(b0, b1) + lcA[0].sum(0)
        acc_f3 = CC(b3, b4) + lcA[1].sum(0)
        acc_g3 = CC(b3, b4) + lcA[2].sum(0)
        acc_f1 = CC(b1, b2) + lcB[0].sum(0)
        acc_g1 = CC(b1, b2) + lcB[1].sum(0)
        acc_f2 = CC(b2, b3) + lcB[2].sum(0)
        acc_g2 = CC(b2, b3) + lcB[3].sum(0)
        acc_f4 = CC(b4, b5) + lcC[0].sum(0)
        acc_g4 = CC(b4, b5) + lcC[1].sum(0)

        def acc_c(lc, blk_col, unit_col, a, ev0, n_ev, b, upper):
            i_apps = ev0 + NK * np.arange(n_ev) + LAG
            i_m = (b - 2) - L
            after = (i_apps[:, None] > i_m[None, :])
            inc = np.where(after, lc[blk_col], lc[unit_col])
            return (Ccum[np.minimum(L, upper), bs] - Ccum[a, bs]) + inc.sum(0)

        acc_c3 = acc_c(lcA, 3, 4, b3, EV0["A"], NEV["A"], b4, b4)
        acc_c4 = acc_c(lcC, 2, 4, b4, EV0["C"], NEV["C"], b5, b5)
        acc_b5 = acc_c(lcC, 3, 4, b5, EV0["C"], NEV["C"], b6, b6)

        def logdot(x, ax, y, ay):
            dv = (x * y).sum(0)
            o = np.full(dv.shape, -np.inf)
            nz = dv > 0
            o[nz] = np.log(dv[nz]) + ax[nz] + ay[nz]
            return o

        def lsum(g, acc):
            return np.log(np.maximum(g.sum(0), 1e-300)) + acc

        lg1 = logdot(g1, acc_g1, a0, acc_a0) - lsum(g1, acc_g1)
        lg2 = logdot(g2, acc_g2, f1, acc_f1) - lsum(g2, acc_g2)
        lg3 = logdot(g3, acc_g3, f2, acc_f2) - lsum(g3, acc_g3)
        lg4 = logdot(g4, acc_g4, f3, acc_f3) - lsum(g4, acc_g4)
        t3 = lg1 + lg2 + logdot(c3, acc_c3, f2, acc_f2)
        t4 = lg1 + lg2 + lg3 + logdot(c4, acc_c4, f3, acc_f3)
        t5 = lg1 + lg2 + lg3 + lg4 + logdot(b5v, acc_b5, f4, acc_f4)
        out[bs] = np.logaddexp(np.logaddexp(t3, t4), t5) \
            + (L + 1) * log_lam
    return out


_CACHED_NC = None
LAST_RESULTS = None


def kernel(feats, mask, transition):
    global _CACHED_NC, LAST_RESULTS
    feats = np.asarray(feats, np.float32)
    mask = np.asarray(mask, np.float32)
    transition = np.asarray(transition, np.float32)
    lengths = mask.sum(axis=0).astype(np.int64)

    in_maps, Ccum, log_lam = _host_prep(feats, transition, lengths)
    if _CACHED_NC is None:
        _CACHED_NC = build_program()
    trace = bool(int(os.environ.get("CRF_TRACE", "0")))
    if trace:
        try:
            import types
            from trn_agent_boot.trn_boot import _ntff_profile_via_ctypes
            if "antenv.axon_hooks" not in sys.modules:
                mm_ = types.ModuleType("antenv.axon_hooks")
                mm_._HOOK = None
                mm_.set_axon_ntff_profile_hook = lambda h: setattr(mm_, "_HOOK", h)
                mm_.get_axon_ntff_profile_hook = lambda: mm_._HOOK
                sys.modules["antenv.axon_hooks"] = mm_
            sys.modules["antenv.axon_hooks"].set_axon_ntff_profile_hook(
                _ntff_profile_via_ctypes("/opt/axon/libaxon_pjrt.so"))
        except Exception as e:
            print(f"ntff hook registration failed: {e}")
    res = run_bass_kernel_spmd(_CACHED_NC, in_maps, core_ids=list(range(NCORES)),
                               trace=trace)
    LAST_RESULTS = res
    out = _reconstruct(res.results, Ccum, lengths, log_lam)
    return out.astype(np.float32)


if __name__ == "__main__":
    feats = np.load("/tmp/in_feats.npy")
    mask = np.load("/tmp/in_mask.npy")
    trans = np.load("/tmp/in_transition.npy")
    got = kernel(feats, mask, trans)
    exp = np.load("/tmp/expected.npy")
    rel = np.abs(got - exp) / np.maximum(1.0, np.abs(exp))
    print("max rel:", rel.max(), "mean:", rel.mean())


# revision 33
# speedup vs baseline: 2.6787x; 1.0220x over previous
"""CRF forward kernel for Trainium2, 8 NeuronCores — K=6 rank-1 segments.

The 1024-step recurrence splits into 6 segments (boundaries
0,170,341,512,682,853,1024); products of ~170 random positive matrices
are rank-1 to machine precision (Perron contraction), so middle segments
factor as T ~ f g^T / d from independently seeded forward/backward
chains.  12 chains pack exactly into 3 bundles of 128 partitions
(4 blocks x 32 tags), each advancing one (matmul -> ef-mul) per step:

  A (170 steps): a0 | f3 | g3 | c3   const=g3.END, marker=c3.END
  B (171 steps): f1 | g1 | f2 | g2   no carriers
  C (171 steps): f4 | g4 | c4 | b5   const=g4.END, markers=c4/b5.END

Carrier rows live in the always-zero rows of backward blocks (W's END
column is forbidden).  Injection needs no extra wiring: a marker value
in a backward block's END row propagates through the natural
W[END,:] = exp(trans[END]) row of the scaled stationary (hence the
(L+1)*log lambda repayment).  Const rows self-normalize via a unit
colsum column in the renorm events; dumped factors make every scale
exactly compensable on the host.

Per step: PE runs 3 matmuls and DVE 3 elementwise muls (both ~90% busy;
GPSIMD cannot read PSUM, ACT has no tensor-tensor op).  ef streams are
fp8 (2.7e-4 rel total vs the 2e-2 gate), chunk-DMA'd up front across
both HWDGE rings, first chunk ahead of everything.  Measured: 137.8us
vs the 354.6us meet-in-the-middle baseline.
"""

import os
import sys

import numpy as np
import ml_dtypes

if "/opt/trn_rl_repo" not in sys.path:
    sys.path.insert(0, "/opt/trn_rl_repo")

import concourse.bass as bass
import concourse.tile as tile
from concourse import bacc, mybir
from concourse.bass_utils import run_bass_kernel_spmd

BF = ml_dtypes.bfloat16
F8 = ml_dtypes.float8_e4m3
S, B, T = 1024, 1024, 32
START, END = T - 2, T - 1
NCORES = 8
FD = 128
NK, LAG = 128, 6
BND = [0, 170, 341, 512, 682, 853, 1024]
QA, QB, QC = 170, 171, 171
EV0 = {"A": 68, "B": 89, "C": 110}
NEV = {k: (q - EV0[k] - 1) // NK + 1
       for k, q in (("A", QA), ("B", QB), ("C", QC))}
NCOL = {"A": 5, "B": 4, "C": 5}
BLK = [slice(32 * k, 32 * k + 32) for k in range(4)]

dt = mybir.dt


def _chunk_bounds(q):
    bounds = [0]
    for inc in (8, 16, 32):
        bounds.append(min(q, bounds[-1] + inc))
    while bounds[-1] < q:
        bounds.append(min(q, bounds[-1] + 128))
    return list(zip(bounds[:-1], bounds[1:]))


def build_program():
    nc = bacc.Bacc("TRN2", target_bir_lowering=False, num_devices=NCORES)

    d = {}
    for k, q in (("A", QA), ("B", QB), ("C", QC)):
        d[f"ef{k}"] = nc.dram_tensor(f"ef{k}", [128, q * FD], dt.float8e4,
                                     kind="ExternalInput")
        d[f"s0{k}"] = nc.dram_tensor(f"s0{k}", [128, FD], dt.bfloat16,
                                     kind="ExternalInput")
        d[f"w{k}"] = nc.dram_tensor(f"w{k}", [128, 128], dt.bfloat16,
                                    kind="ExternalInput")
        d[f"ob{k}"] = nc.dram_tensor(f"ob{k}", [128, NCOL[k]], dt.bfloat16,
                                     kind="ExternalInput")
        d[f"oc{k}"] = nc.dram_tensor(f"oc{k}", [NCOL[k], 128], dt.bfloat16,
                                     kind="ExternalInput")
        d[f"rd{k}"] = nc.dram_tensor(f"rd{k}", [NCOL[k], NEV[k] * FD],
                                     dt.bfloat16, kind="ExternalOutput")
    d["qpk"] = nc.dram_tensor("qpk", [128, 3 * FD], dt.bfloat16,
                              kind="ExternalOutput")

    with tile.TileContext(nc) as tc:
        with (
            tc.tile_pool(name="singles", bufs=1) as singles,
            tc.tile_pool(name="efpool", bufs=1) as efpool,
            tc.tile_pool(name="small", bufs=2) as small,
            tc.tile_pool(name="stA", bufs=3) as stA,
            tc.tile_pool(name="stB", bufs=3) as stB,
            tc.tile_pool(name="stC", bufs=3) as stC,
            tc.tile_pool(name="psA", bufs=2, space="PSUM") as psA,
            tc.tile_pool(name="psB", bufs=2, space="PSUM") as psB,
            tc.tile_pool(name="psC", bufs=2, space="PSUM") as psC,
            tc.tile_pool(name="psE", bufs=1, space="PSUM") as psE,
        ):
            t = {}
            ring = {"A": nc.sync, "B": nc.scalar, "C": nc.sync}
            ring2 = {"A": nc.scalar, "B": nc.sync, "C": nc.scalar}
            # first ef chunk ahead of everything so compute starts early
            spans = {k: _chunk_bounds(q)
                     for k, q in (("A", QA), ("B", QB), ("C", QC))}
            chunks = {}
            efpool_tiles = {}
            for k in ("A", "B", "C"):
                lo, hi = spans[k][0]
                cw = hi - lo
                tl = efpool.tile([128, cw * FD], dt.float8e4,
                                 tag=f"ef{k}0", name=f"ef{k}_0")
                ring[k].dma_start(out=tl[0:64, :],
                                  in_=d[f"ef{k}"].ap()[0:64, lo * FD:hi * FD])
                ring2[k].dma_start(out=tl[64:128, :],
                                   in_=d[f"ef{k}"].ap()[64:128,
                                                        lo * FD:hi * FD])
                chunks[(k, 0)] = (tl, lo, hi)
            for k in ("A", "B", "C"):
                t[f"w{k}"] = singles.tile([128, 128], dt.bfloat16,
                                          tag=f"w{k}", name=f"w{k}_t")
                t[f"ob{k}"] = singles.tile([128, NCOL[k]], dt.bfloat16,
                                           tag=f"ob{k}", name=f"ob{k}_t")
                t[f"oc{k}"] = singles.tile([NCOL[k], 128], dt.bfloat16,
                                           tag=f"oc{k}", name=f"oc{k}_t")
                r = ring[k]
                r.dma_start(out=t[f"w{k}"], in_=d[f"w{k}"].ap())
                r.dma_start(out=t[f"ob{k}"], in_=d[f"ob{k}"].ap())
                r.dma_start(out=t[f"oc{k}"], in_=d[f"oc{k}"].ap())
                t[f"rb{k}"] = singles.tile([NCOL[k], NEV[k] * FD],
                                           dt.bfloat16, tag=f"rb{k}",
                                           name=f"rb{k}")

            pools = {"A": stA, "B": stB, "C": stC}
            cur = {}
            for k in ("A", "B", "C"):
                cur[k] = pools[k].tile([128, FD], dt.bfloat16, tag="s",
                                       name=f"s{k}_0")
                ring2[k].dma_start(out=cur[k], in_=d[f"s0{k}"].ap())

            nch = len(spans["A"])
            for ch in range(1, nch):
                for k in ("A", "B", "C"):
                    if ch >= len(spans[k]):
                        continue
                    lo, hi = spans[k][ch]
                    cw = hi - lo
                    tl = efpool.tile([128, cw * FD], dt.float8e4,
                                     tag=f"ef{k}{ch}", name=f"ef{k}_{ch}")
                    ring[k].dma_start(
                        out=tl[0:64, :],
                        in_=d[f"ef{k}"].ap()[0:64, lo * FD:hi * FD])
                    ring2[k].dma_start(
                        out=tl[64:128, :],
                        in_=d[f"ef{k}"].ap()[64:128, lo * FD:hi * FD])
                    chunks[(k, ch)] = (tl, lo, hi)

            pend = {"A": {}, "B": {}, "C": {}}
            qpack = singles.tile([128, 3 * FD], dt.bfloat16, tag="qpack",
                                 name="qpack")

            def event(k, i, mul_engine):
                e = (i - EV0[k]) // NK
                ncol = NCOL[k]
                psc = psE.tile([5, FD], dt.float32, tag="psC",
                               name=f"psC{k}_{i}")[0:ncol, :]
                nc.tensor.matmul(psc, t[f"ob{k}"], cur[k], start=True,
                                 stop=True)
                rf = small.tile([5, FD], dt.float32, tag="rf",
                                name=f"rf{k}_{i}")[0:ncol, :]
                nc.vector.reciprocal_approx_fast(out=rf, in_=psc)
                rsb = t[f"rb{k}"][:, e * FD:(e + 1) * FD]
                nc.vector.tensor_copy(rsb, rf)
                if e == NEV[k] - 1:     # dump factors mid-loop, off the tail
                    ring2[k].dma_start(out=d[f"rd{k}"].ap(), in_=t[f"rb{k}"])
                q = {"A": QA, "B": QB, "C": QC}[k]
                if i + LAG < q:
                    pend[k][i + LAG] = rsb

            def step(k, i, ch, lo, tl, ps_pool, mul_engine):
                if i >= EV0[k] and (i - EV0[k]) % NK == 0:
                    event(k, i, mul_engine)
                csl = slice((i - lo) * FD, (i - lo) * FD + FD)
                esl = tl[:, csl]
                if i in pend[k]:
                    rsb = pend[k].pop(i)
                    psr = psE.tile([128, FD], dt.float32, tag="psR",
                                   name=f"psR{k}_{i}")
                    nc.tensor.matmul(psr, t[f"oc{k}"], rsb, start=True,
                                     stop=True)
                    efx = small.tile([128, FD], dt.bfloat16, tag="efx",
                                     name=f"efx{k}_{i}")
                    nc.vector.tensor_mul(efx, psr, esl)
                    esl = efx
                ps = ps_pool.tile([128, FD], dt.float32, tag="ps",
                                  name=f"ps{k}_{i}")
                nc.tensor.matmul(ps, t[f"w{k}"], cur[k], start=True,
                                 stop=True)
                q = {"A": QA, "B": QB, "C": QC}[k]
                if i == q - 1:          # final state -> packed output tile
                    ki = {"A": 0, "B": 1, "C": 2}[k]
                    nxt = qpack[:, ki * FD:(ki + 1) * FD]
                else:
                    nxt = pools[k].tile([128, FD], dt.bfloat16, tag="s",
                                        name=f"s{k}_{i + 1}")
                mul_engine.tensor_mul(nxt, ps, esl)
                cur[k] = nxt

            chi = {"A": 0, "B": 0, "C": 0}
            for i in range(QC):
                for k, q, psp, eng in (("A", QA, psA, nc.vector),
                                       ("B", QB, psB, nc.vector),
                                       ("C", QC, psC, nc.vector)):
                    if i >= q:
                        continue
                    if i >= spans[k][chi[k]][1]:
                        chi[k] += 1
                    tl, lo, hi = chunks[(k, chi[k])]
                    step(k, i, chi[k], lo, tl, psp, eng)

            nc.sync.dma_start(out=d["qpk"].ap(), in_=qpack)

    nc.finalize()
    return nc


def _host_prep(feats, transition, lengths):
    b_tot = feats.shape[1]
    n_cores = b_tot // FD
    b0, b1, b2, b3, b4, b5, b6 = BND
    c_pre = feats.max(axis=2)                                # (S, B)
    Ccum = np.vstack([np.zeros((1, b_tot), np.float64),
                      np.cumsum(c_pre.astype(np.float64), 0)])
    efq = np.exp(feats - c_pre[:, :, None]).astype(np.float32)   # (S,B,T)

    ef_mean = efq.mean(axis=(0, 1)).astype(np.float64)
    Wd = np.exp(transition.astype(np.float64))
    lam = np.abs(np.linalg.eigvals(ef_mean[:, None] * Wd)).max()
    log_lam = float(np.log(lam))
    Ws = Wd / lam
    lhsF = Ws.T
    lhsB = Ws
    eT = np.exp(transition[END].astype(np.float64))

    def bundle_w(kinds, const_blk, mark_blks):
        Wm = np.zeros((128, 128))
        for k, kind in enumerate(kinds):
            Wm[BLK[k], BLK[k]] = lhsF if kind == 'F' else lhsB
        if const_blk is not None:
            ce = 32 * const_blk + END
            Wm[ce, BLK[const_blk]] = 0.0
            Wm[ce, ce] = 1.0
            for mb in mark_blks:
                Wm[ce, 32 * mb + END] = 1.0
        return Wm.astype(BF)

    def bundle_oboc(const_blk, mark_blks, guard_cblks, ncol):
        ob = np.zeros((128, ncol), np.float32)
        oc = np.zeros((ncol, 128), np.float32)
        carrier = []
        if const_blk is not None:
            carrier.append(32 * const_blk + END)
            carrier += [32 * mb + END for mb in mark_blks]
        for k in range(4):
            rows = [r for r in range(32 * k, 32 * k + 32) if r not in carrier]
            ob[rows, k] = 1.0
            oc[k, rows] = 1.0
        if const_blk is not None:
            ce = 32 * const_blk + END
            ob[ce, ncol - 1] = 1.0
            oc[ncol - 1, ce] = 1.0
            for mb in mark_blks:
                oc[ncol - 1, 32 * mb + END] = 1.0
            for cb in guard_cblks:
                ob[ce, cb] = 1.0
        return ob.astype(BF), oc.astype(BF)

    wA = bundle_w(['F', 'F', 'B', 'B'], 2, [3])
    wB = bundle_w(['F', 'B', 'F', 'B'], None, [])
    wC = bundle_w(['F', 'B', 'B', 'B'], 1, [2, 3])
    obA, ocA = bundle_oboc(2, [3], [3], 5)
    obB, ocB = bundle_oboc(None, [], [], 4)
    obC, ocC = bundle_oboc(1, [2, 3], [2, 3], 5)

    Lall = lengths.astype(int)
    in_maps = []
    for core in range(n_cores):
        sl = slice(core * FD, (core + 1) * FD)
        E = np.ascontiguousarray(efq[:, sl, :].transpose(0, 2, 1))  # (S,T,FD)
        Lc = Lall[sl]
        mark = np.zeros((S + 1, FD), np.float32)
        mark[Lc, np.arange(FD)] = 1.0

        def fcols(a, b, q):
            return E[a:b].transpose(1, 0, 2)                 # (T, q, FD)

        def bcols(a, b, q, mark_lo=None, mark_hi=None, zero_end=False):
            ts = b - 2 - np.arange(q)
            out = np.stack([E[tt] if tt >= a else np.ones((T, FD), np.float32)
                            for tt in ts], axis=1)
            if zero_end:
                out[END] = 1.0
            if mark_lo is not None:
                out[END] = np.stack(
                    [mark[tt] if mark_lo <= tt <= mark_hi
                     else np.zeros(FD, np.float32) for tt in ts], axis=0)
            return out

        efA = np.concatenate([
            fcols(b0, b1, QA), fcols(b3, b4, QA),
            bcols(b3, b4, QA, zero_end=True),
            bcols(b3, b4, QA, mark_lo=b3, mark_hi=b4 - 2)], axis=0)
        efB = np.concatenate([
            fcols(b1, b2, QB), bcols(b1, b2, QB),
            fcols(b2, b3, QB), bcols(b2, b3, QB)], axis=0)
        efC = np.concatenate([
            fcols(b4, b5, QC), bcols(b4, b5, QC, zero_end=True),
            bcols(b4, b5, QC, mark_lo=b4, mark_hi=b5 - 2),
            bcols(b5, b6, QC, mark_lo=b5, mark_hi=b6 - 2)], axis=0)

        def bseed(b):
            s = E[b - 1].copy()
            s[END] = 0.0
            return s

        A0 = np.zeros((128, FD), np.float32)
        A0[START] = 1.0
        A0[BLK[1]] = 1.0
        A0[BLK[2]] = bseed(b4)
        A0[64 + END] = 1.0
        A0[96 + END] = mark[b4 - 1]

        B0 = np.zeros((128, FD), np.float32)
        B0[BLK[0]] = 1.0
        B0[BLK[1]] = bseed(b2)
        B0[BLK[2]] = 1.0
        B0[BLK[3]] = bseed(b3)

        C0 = np.zeros((128, FD), np.float32)
        C0[BLK[0]] = 1.0
        C0[BLK[1]] = bseed(b5)
        C0[32 + END] = 1.0
        C0[64 + END] = mark[b5 - 1]
        C0[BLK[3]] = (eT / lam)[:, None].astype(np.float32) \
            * mark[b6][None, :] * E[b6 - 1]
        C0[96 + END] = mark[b6 - 1]

        in_maps.append({
            "efA": np.ascontiguousarray(efA).reshape(128, QA * FD).astype(F8),
            "efB": np.ascontiguousarray(efB).reshape(128, QB * FD).astype(F8),
            "efC": np.ascontiguousarray(efC).reshape(128, QC * FD).astype(F8),
            "s0A": A0.astype(BF), "s0B": B0.astype(BF), "s0C": C0.astype(BF),
            "wA": wA, "wB": wB, "wC": wC,
            "obA": obA, "obB": obB, "obC": obC,
            "ocA": ocA, "ocB": ocB, "ocC": ocC,
        })
    return in_maps, Ccum, log_lam


def _reconstruct(results, Ccum, lengths, log_lam):
    n_cores = len(results)
    b0, b1, b2, b3, b4, b5, b6 = BND
    out = np.zeros(n_cores * FD, np.float64)
    for core in range(n_cores):
        res = results[core]
        qpk = res["qpk"].astype(np.float64)
        Af = qpk[:, 0:FD]
        Bf = qpk[:, FD:2 * FD]
        Cf = qpk[:, 2 * FD:3 * FD]
        lcA = -np.log(np.maximum(res["rdA"].astype(np.float64)
                                 .reshape(5, NEV["A"], FD), 1e-300))
        lcB = -np.log(np.maximum(res["rdB"].astype(np.float64)
                                 .reshape(4, NEV["B"], FD), 1e-300))
        lcC = -np.log(np.maximum(res["rdC"].astype(np.float64)
                                 .reshape(5, NEV["C"], FD), 1e-300))
        bs = core * FD + np.arange(FD)
        L = lengths[bs].astype(int)

        def blk(Xf, k, zero_end=False):
            v = Xf[BLK[k]].copy()
            if zero_end:
                v[END] = 0.0
            return v

        a0 = blk(Af, 0)
        f3 = blk(Af, 1)
        g3 = blk(Af, 2, True)
        c3 = blk(Af, 3, True)
        f1 = blk(Bf, 0)
        g1 = blk(Bf, 1)
        f2 = blk(Bf, 2)
        g2 = blk(Bf, 3)
        f4 = blk(Cf, 0)
        g4 = blk(Cf, 1, True)
        c4 = blk(Cf, 2, True)
        b5v = blk(Cf, 3, True)

        def CC(a, b):
            return Ccum[b, bs] - Ccum[a, bs]

        acc_a0 = CC(b0, b1) + lcA[0].sum(0)
        acc_f3 = CC(b3, b4) + lcA[1].sum(0)
        acc_g3 = CC(b3, b4) + lcA[2].sum(0)
        acc_f1 = CC(b1, b2) + lcB[0].sum(0)
        acc_g1 = CC(b1, b2) + lcB[1].sum(0)
        acc_f2 = CC(b2, b3) + lcB[2].sum(0)
        acc_g2 = CC(b2, b3) + lcB[3].sum(0)
        acc_f4 = CC(b4, b5) + lcC[0].sum(0)
        acc_g4 = CC(b4, b5) + lcC[1].sum(0)

        def acc_c(lc, blk_col, unit_col, a, ev0, n_ev, b, upper):
            i_apps = ev0 + NK * np.arange(n_ev) + LAG
            i_m = (b - 2) - L
            after = (i_apps[:, None] > i_m[None, :])
            inc = np.where(after, lc[blk_col], lc[unit_col])
            return (Ccum[np.minimum(L, upper), bs] - Ccum[a, bs]) + inc.sum(0)

        acc_c3 = acc_c(lcA, 3, 4, b3, EV0["A"], NEV["A"], b4, b4)
        acc_c4 = acc_c(lcC, 2, 4, b4, EV0["C"], NEV["C"], b5, b5)
        acc_b5 = acc_c(lcC, 3, 4, b5, EV0["C"], NEV["C"], b6, b6)

        def logdot(x, ax, y, ay):
            dv = (x * y).sum(0)
            o = np.full(dv.shape, -np.inf)
            nz = dv > 0
            o[nz] = np.log(dv[nz]) + ax[nz] + ay[nz]
            return o

        def lsum(g, acc):
            return np.log(np.maximum(g.sum(0), 1e-300)) + acc

        lg1 = logdot(g1, acc_g1, a0, acc_a0) - lsum(g1, acc_g1)
        lg2 = logdot(g2, acc_g2, f1, acc_f1) - lsum(g2, acc_g2)
        lg3 = logdot(g3, acc_g3, f2, acc_f2) - lsum(g3, acc_g3)
        lg4 = logdot(g4, acc_g4, f3, acc_f3) - lsum(g4, acc_g4)
        t3 = lg1 + lg2 + logdot(c3, acc_c3, f2, acc_f2)
        t4 = lg1 + lg2 + lg3 + logdot(c4, acc_c4, f3, acc_f3)
        t5 = lg1 + lg2 + lg3 + lg4 + logdot(b5v, acc_b5, f4, acc_f4)
        out[bs] = np.logaddexp(np.logaddexp(t3, t4), t5) \
            + (L + 1) * log_lam
    return out


_CACHED_NC = None
LAST_RESULTS = None


def kernel(feats, mask, transition):
    global _CACHED_NC, LAST_RESULTS
    feats = np.asarray(feats, np.float32)
    mask = np.asarray(mask, np.float32)
    transition = np.asarray(transition, np.float32)
    lengths = mask.sum(axis=0).astype(np.int64)

    in_maps, Ccum, log_lam = _host_prep(feats, transition, lengths)
    if _CACHED_NC is None:
        _CACHED_NC = build_program()
    trace = bool(int(os.environ.get("CRF_TRACE", "0")))
    if trace:
        try:
            import types
            from trn_agent_boot.trn_boot import _ntff_profile_via_ctypes
            if "antenv.axon_hooks" not in sys.modules:
                mm_ = types.ModuleType("antenv.axon_hooks")
                mm_._HOOK = None
                mm_.set_axon_ntff_profile_hook = lambda h: setattr(mm_, "_HOOK", h)
                mm_.get_axon_ntff_profile_hook = lambda: mm_._HOOK
                sys.modules["antenv.axon_hooks"] = mm_
            sys.modules["antenv.axon_hooks"].set_axon_ntff_profile_hook(
                _ntff_profile_via_ctypes("/opt/axon/libaxon_pjrt.so"))
        except Exception as e:
            print(f"ntff hook registration failed: {e}")
    res = run_bass_kernel_spmd(_CACHED_NC, in_maps, core_ids=list(range(NCORES)),
                               trace=trace)
    LAST_RESULTS = res
    out = _reconstruct(res.results, Ccum, lengths, log_lam)
    return out.astype(np.float32)


if __name__ == "__main__":
    feats = np.load("/tmp/in_feats.npy")
    mask = np.load("/tmp/in_mask.npy")
    trans = np.load("/tmp/in_transition.npy")
    got = kernel(feats, mask, trans)
    exp = np.load("/tmp/expected.npy")
    rel = np.abs(got - exp) / np.maximum(1.0, np.abs(exp))
    print("max rel:", rel.max(), "mean:", rel.mean())
